# revision 23
# baseline (speedup 1.0000x reference)
"""GAT (3-layer, PyG-style) on 8 Trainium2 NeuronCores via Bass/Tile.

Strategy (dst-sharded graph parallel, v2):
  - Nodes permuted for per-tile load balance, sharded 8 ways by destination;
    edges partitioned by (core, dst tile), split lo/hi by source position
    (int16 gather limit), chunked to 128 edges.
  - Per layer a node table [N, 256] bf16 holds the head-features ROTATED so
    that slots 0,1 of each head are exactly al_src / al_dst (basis matrix M
    with columns [a_src | a_dst | orthonormal complement] folded into W).
    512B rows hit the DMA descriptor cost floor; the inverse rotation is one
    extra 128-col matmul per half in the epilogue.
  - Self-loops are a constant-identity chunk fed from the core's own rows
    (no gather, no one-hot bytes).
  - idx16 | S | ST packed into one blob DMA per 4-tile group; softmax is
    pre-normalized: denominators first (4-col matmuls), 1/den expanded per
    edge via the already-loaded ST, then alpha replaces e everywhere.
  - alpha replicated into bf16 pairs so the big alpha*h multiply runs in the
    DVE 2x_1p mode; epilogue matmuls run bf16/f32r at 1 cycle/column.
"""
import numpy as np
import ml_dtypes
from contextlib import ExitStack

import concourse.bass as bass
import concourse.tile as tile
from concourse import bacc, mybir
from concourse import bass_utils
from concourse.masks import make_identity

P = 128
N_NODES = 50000
N_EDGES = 650000
NEG_SLOPE = 0.2
N_CORES = 8
NS = N_NODES // N_CORES            # 6250 nodes per shard
NT = (NS + P - 1) // P             # 49 dst tiles per core
NSP = NT * P                       # padded shard nodes (6272)
HALF = 32768                       # int16 gather limit -> low/high split
ROW = 256                          # bf16 slots per table row (512B), layers 1,2
ROW3 = 128                         # bf16 slots per layer-3 table row (256B)
F = 256                            # feature width (H*C)
H = 4
GT = 4                             # tiles per group
GMAX = 8                           # chunks per dma_gather (1024-desc ucode cap)

FP8 = mybir.dt.float8e4
BF16 = mybir.dt.bfloat16
FP16 = mybir.dt.float16
F32 = mybir.dt.float32
F32R = mybir.dt.float32r
I16 = mybir.dt.int16
U8 = mybir.dt.uint8

SLOTS01 = [h * 64 + s for h in range(H) for s in range(2)]


# ----------------------------------------------------------------- host prep

def _wrap16(idx_flat):
    """[n] int array -> [128, n//16] int16 (16-partition wrap, replicated)."""
    n = len(idx_flat)
    a = np.asarray(idx_flat, dtype=np.int16).reshape(n // 16, 16).T
    return np.tile(a, (8, 1))


def _balance_perm(dst):
    """Permute nodes so per-(core,tile) edge counts are balanced.

    Returns pos_of_node [N] (permuted global position: core*NS + local)."""
    deg = np.bincount(dst, minlength=N_NODES)
    order = np.argsort(-deg, kind="stable")
    NB = N_CORES * NT                       # bins; bin b -> (tile b//8, core b%8)
    cap = np.empty(NB, np.int64)
    for t in range(NT):
        c = 128 if t < NT - 1 else NS - 128 * (NT - 1)
        cap[t * N_CORES:(t + 1) * N_CORES] = c
    # snake deal by descending degree over non-full bins
    snake = []
    for r in range(128):
        idxs = [b for b in range(NB) if cap[b] > r]
        if r % 2:
            idxs = idxs[::-1]
        snake.extend(idxs)
    assert len(snake) == N_NODES
    fill = np.zeros(NB, np.int64)
    pos_of_node = np.empty(N_NODES, np.int64)
    for i, n in enumerate(order):
        b = snake[i]
        t, c = b // N_CORES, b % N_CORES
        pos_of_node[n] = c * NS + t * 128 + fill[b]
        fill[b] += 1
    return pos_of_node


def build_schedule(src, dst, pos_of_node):
    """Edges by (core, tile, half); per-tile chunk counts maxed over cores;
    blob = [idx16 | S | ST] per 4-tile group."""
    spos = pos_of_node[src]
    dpos = pos_of_node[dst]
    core_of = dpos // NS
    local = dpos % NS
    tile_of = local // 128
    dslot = local % 128
    is_hi = spos >= HALF

    # per (core, tile): lo and hi edge arrays (srcidx, dslot)
    per = {}
    okey = core_of * (NT * 2) + tile_of * 2 + is_hi
    order = np.argsort(okey, kind="stable")
    so, do_, ko = spos[order], dslot[order], okey[order]
    bounds = np.searchsorted(ko, np.arange(N_CORES * NT * 2 + 1))
    cnt = np.zeros((N_CORES, NT, 2), np.int64)
    for c in range(N_CORES):
        for t in range(NT):
            for hh in range(2):
                k = c * (NT * 2) + t * 2 + hh
                b0, b1 = bounds[k], bounds[k + 1]
                per[(c, t, hh)] = (so[b0:b1] - (HALF if hh else 0), do_[b0:b1])
                cnt[c, t, hh] = b1 - b0

    LCH = np.maximum(-(-cnt[:, :, 0].max(0) // 128), 0)
    HCH = np.maximum(-(-cnt[:, :, 1].max(0) // 128), 0)

    # groups of GT tiles
    groups = []
    t0 = 0
    while t0 < NT:
        gt = min(GT, NT - t0)
        groups.append((t0, gt))
        t0 += gt

    # chunk layout per group: [tile-major lo][tile-major hi]
    meta = []
    TOTCH = int(LCH.sum() + HCH.sum())
    idx_all = np.zeros((N_CORES, P, TOTCH * 8), np.int16)
    one = ml_dtypes.float8_e4m3(1.0)
    S = np.zeros((N_CORES, P, TOTCH, P), ml_dtypes.float8_e4m3)
    ST = np.zeros((N_CORES, P, TOTCH, P), ml_dtypes.float8_e4m3)
    ch = 0
    for (t0, gt) in groups:
        lw = int(LCH[t0:t0 + gt].sum())
        hw = int(HCH[t0:t0 + gt].sum())
        tinfo = []
        lo_off = 0
        hi_off = 0
        for j in range(gt):
            t = t0 + j
            tinfo.append((int(lo_off), int(LCH[t]), int(hi_off), int(HCH[t])))
            lo_off += int(LCH[t])
            hi_off += int(HCH[t])
        meta.append(dict(t0=t0, gt=gt, lw=lw, hw=hw, ch0=ch, tinfo=tinfo))
        for c in range(N_CORES):
            for j in range(gt):
                t = t0 + j
                for hh in range(2):
                    nch = int((LCH[t], HCH[t])[hh])
                    if nch == 0:
                        continue
                    base = ch + (tinfo[j][0] if hh == 0 else lw + tinfo[j][2])
                    es, ed = per[(c, t, hh)]
                    n = nch * 128
                    e_pad = np.zeros(n, np.int64)
                    e_pad[:len(es)] = es
                    if len(es):
                        k = np.arange(len(es))
                        S[c, k % P, base + k // P, ed] = one
                        ST[c, ed, base + k // P, k % P] = one
                    idx_all[c, :, base * 8:(base + nch) * 8] = _wrap16(e_pad)
        ch += lw + hw

    # blob: per group [idx (nch*16B) | S (nch*128B) | ST (nch*128B)]
    blob_parts = [[] for _ in range(N_CORES)]
    off = 0
    for m in meta:
        c0, nch = m["ch0"], m["lw"] + m["hw"]
        m["blob_off"] = off
        m["nch"] = nch
        off += nch * (16 + 128 + 128)
        for c in range(N_CORES):
            blob_parts[c].append(idx_all[c, :, c0 * 8:(c0 + nch) * 8].view(np.uint8))
            blob_parts[c].append(S[c, :, c0:c0 + nch, :].reshape(P, nch * 128).view(np.uint8))
            blob_parts[c].append(ST[c, :, c0:c0 + nch, :].reshape(P, nch * 128).view(np.uint8))
    blob = np.stack([np.concatenate(blob_parts[c], axis=1) for c in range(N_CORES)])
    return dict(meta=meta, TOTB=off, TOTCH=TOTCH, blob=np.ascontiguousarray(blob),
                LCH=LCH, HCH=HCH)


def _rot_M(a_src, a_dst):
    """Per-head invertible M with columns [a_src | a_dst | complement]."""
    Hh, C = a_src.shape
    blocks = []
    for h in range(Hh):
        pair = np.stack([a_src[h], a_dst[h]], axis=1).astype(np.float64)
        Q, _ = np.linalg.qr(pair, mode="complete")
        M = np.concatenate([pair, Q[:, 2:]], axis=1)
        blocks.append(M)
    return blocks


def _blockdiag(blocks):
    n = sum(b.shape[0] for b in blocks)
    m = sum(b.shape[1] for b in blocks)
    out = np.zeros((n, m), np.float64)
    r = c = 0
    for b in blocks:
        out[r:r + b.shape[0], c:c + b.shape[1]] = b
        r += b.shape[0]
        c += b.shape[1]
    return out


# ------------------------------------------------------------- bass builders

class Prog:
    def __init__(self):
        self.nc = bacc.Bacc("TRN2", target_bir_lowering=False, debug=False,
                            num_devices=N_CORES,
                            dynamic_dma_scratch_size=32768)
        self.in_aps = {}
        self.out_aps = {}

    def inp(self, name, shape, dt):
        ap = self.nc.dram_tensor(name, list(shape), dt, kind="ExternalInput").ap()
        self.in_aps[name] = ap
        return ap

    def out(self, name, shape, dt):
        ap = self.nc.dram_tensor(name, list(shape), dt, kind="ExternalOutput").ap()
        self.out_aps[name] = ap
        return ap


def build_launch_A():
    """x_shard @ W1rot -> table1 rows (rotated h1, bf16)."""
    pr = Prog()
    nc = pr.nc
    x = pr.inp("x", [NSP, P], F32)
    w1 = pr.inp("w1", [P, F], F32)
    tab = pr.out("tab", [NSP, ROW], FP16)
    with tile.TileContext(nc) as tc, ExitStack() as ctx:
        sb = ctx.enter_context(tc.tile_pool(name="sb", bufs=5))
        ps = ctx.enter_context(tc.tile_pool(name="ps", bufs=4, space="PSUM"))
        cpool = ctx.enter_context(tc.tile_pool(name="cp", bufs=1))
        ident = cpool.tile([P, P], F32)
        make_identity(nc, ident[:])
        w1t = cpool.tile([P, F], F32)
        nc.sync.dma_start(w1t[:], w1)
        B4 = 7
        for t0 in range(0, NT, B4):
            nb = min(B4, NT - t0)
            xt = sb.tile([P, B4, P], F32, tag="xt")
            nc.sync.dma_start(
                xt[:, 0:nb, :],
                x[t0 * P:(t0 + nb) * P, :].rearrange("(b p) f -> p b f", p=P))
            trow = sb.tile([P, B4, ROW], FP16, tag="trow")
            for j in range(nb):
                xT_ps = ps.tile([P, P], F32, space="PSUM", tag="xT")
                nc.tensor.transpose(xT_ps[:], xt[:, j, :], ident[:])
                xT = sb.tile([P, P], F32, tag="xTs")
                nc.scalar.copy(xT[:], xT_ps[:])
                h_ps = ps.tile([P, F], F32, space="PSUM", tag="hps")
                nc.tensor.matmul(h_ps[:], lhsT=xT[:], rhs=w1t[:],
                                 start=True, stop=True)
                nc.scalar.copy(trow[:, j, :], h_ps[:])
            nc.scalar.dma_start(
                tab[t0 * P:(t0 + nb) * P, :].rearrange("(b p) f -> p b f", p=P),
                trow[:, 0:nb, :])
    nc.compile()
    return pr


def build_launch_agg(sch, layer, b3=0.0, inv_as3=1.0):
    """layer=1: L1 agg -> table2; layer=2: L2 agg -> table3; layer=3: out."""
    pr = Prog()
    nc = pr.nc
    last = layer == 3
    row = ROW3 if last else ROW
    nad = 1 if last else H
    table = pr.inp("table", [N_NODES, row], FP16)
    mytab = pr.inp("mytab", [NSP, row], FP16)
    blob_in = pr.inp("blob", [P, sch["TOTB"]], U8)
    if layer == 1:
        naug = F
        nwc = 8
        waug = pr.inp("waug", [F, naug], FP16)
        tabout = pr.out("tabout", [NSP, ROW], FP16)
    elif layer == 2:
        naug = 2
        nwc = 2
        waug = pr.inp("waug", [F, naug], FP16)
        tabout = pr.out("tabout", [NSP, ROW3], FP16)
    else:
        outv = pr.out("outv", [NSP, 1], F32)
    if not last:
        wc = pr.inp("wc", [P, nwc], F32)       # (ones@Waug)[slots], replicated
        minv = pr.inp("minv", [P, 2, P], FP16)  # per-half blockdiag(Minv)
        bias = pr.inp("bias", [P, 2], F32)
        nbias = pr.inp("nbias", [P, 2], F32)

    with tile.TileContext(nc) as tc, ExitStack() as ctx:
        sb = ctx.enter_context(tc.tile_pool(name="sb", bufs=2))
        sbg = ctx.enter_context(tc.tile_pool(name="sbg", bufs=2))
        ps = ctx.enter_context(tc.tile_pool(name="ps", bufs=2, space="PSUM"))
        psb = ctx.enter_context(tc.tile_pool(name="psb", bufs=1, space="PSUM"))
        # PSUM banks: combo(2) + agg(2) + yT(2) + xT(1) + hps(1) = 8
        cpool = ctx.enter_context(tc.tile_pool(name="cp", bufs=1))
        ident8 = cpool.tile([P, P], FP8)
        make_identity(nc, ident8[:])
        if not last:
            identb = cpool.tile([P, P], FP16)
            make_identity(nc, identb[:])
            waug_t = cpool.tile([P, F // P, naug], FP16, tag="waug")
            for k in range(F // P):
                nc.sync.dma_start(waug_t[:, k, :], waug[k * P:(k + 1) * P, :])
            wc_t = cpool.tile([P, nwc], F32, tag="wc")
            nc.sync.dma_start(wc_t[:], wc)
            minv_t = cpool.tile([P, 2, P], FP16, tag="minv")
            nc.sync.dma_start(minv_t[:], minv)
            bias_t = cpool.tile([P, 2], F32, tag="bias")
            nc.sync.dma_start(bias_t[:], bias)
            nbias_t = cpool.tile([P, 2], F32, tag="nbias")
            nc.sync.dma_start(nbias_t[:], nbias)
        else:
            obuf = cpool.tile([P, NT], F32, tag="obuf")

        for m in sch["meta"]:
            t0, gt, lw, hw, nch = m["t0"], m["gt"], m["lw"], m["hw"], m["nch"]
            boff = m["blob_off"]
            tinfo = m["tinfo"]
            # ---- loads
            blob_t = sbg.tile([P, nch * 272], U8, tag="blob")
            nc.sync.dma_start(blob_t[:], blob_in[:, boff:boff + nch * 272])
            idx_v = blob_t[:, 0:nch * 16].bitcast(I16)
            s_v = blob_t[:, nch * 16:nch * 144].bitcast(FP8).rearrange(
                "p (c e) -> p c e", e=P)
            st_v = blob_t[:, nch * 144:nch * 272].bitcast(FP8).rearrange(
                "p (c e) -> p c e", e=P)
            selfr = sb.tile([P, gt, row], FP16, tag="selfr")
            nc.sync.dma_start(
                selfr[:],
                mytab[t0 * P:(t0 + gt) * P, :].rearrange("(b p) f -> p b f", p=P))
            # ---- gathers
            g_lo = g_hi = None
            if lw:
                g_lo = sbg.tile([P, lw, row], FP16, tag="glo")
                for w0 in range(0, lw, GMAX):
                    w1_ = min(w0 + GMAX, lw)
                    nc.gpsimd.dma_gather(
                        out_ap=g_lo[:, w0:w1_, :], in_ap=table,
                        idxs_ap=idx_v[:, w0 * 8:w1_ * 8],
                        num_idxs=(w1_ - w0) * P, num_idxs_reg=(w1_ - w0) * P,
                        elem_size=row)
            if hw:
                g_hi = sbg.tile([P, hw, row], FP16, tag="ghi")
                for w0 in range(0, hw, GMAX):
                    w1_ = min(w0 + GMAX, hw)
                    nc.gpsimd.dma_gather(
                        out_ap=g_hi[:, w0:w1_, :], in_ap=table[HALF:, :],
                        idxs_ap=idx_v[:, (lw + w0) * 8:(lw + w1_) * 8],
                        num_idxs=(w1_ - w0) * P, num_idxs_reg=(w1_ - w0) * P,
                        elem_size=row)

            # ---- as/ad of own nodes (slots 0,1 per head)
            if not last:
                selfr_v = selfr[:].rearrange("p b (h c) -> p b h c", h=H)
                asad = sb.tile([P, gt, H, 2], FP16, tag="asad")
                nc.vector.tensor_copy(asad[:], selfr_v[:, :, :, 0:2])
                as_own, ad_own = asad[:, :, :, 0], asad[:, :, :, 1]
            else:
                as_own, ad_own = selfr[:, :, 0:1], selfr[:, :, 1:2]

            # ---- a_dst expansion (zps) per chunk; combo also holds
            # the r-expansion (zps2) and per-tile denominators in one bank
            combo = ps.tile([P, 2 * nch + gt, nad], F32, space="PSUM", tag="combo")
            zps = combo[:, 0:nch, :]
            zps2 = combo[:, nch:2 * nch, :]
            den = combo[:, 2 * nch:2 * nch + gt, :]
            for j in range(gt):
                lo0, lcnt, hi0, hcnt = tinfo[j]
                adt = sb.tile([P, nad], FP16, tag="adt")
                nc.vector.tensor_copy(adt[:], ad_own[:, j, :])
                for cj in (list(range(lo0, lo0 + lcnt))
                           + list(range(lw + hi0, lw + hi0 + hcnt))):
                    nc.tensor.matmul(zps[:, cj, :], lhsT=st_v[:, cj, :],
                                     rhs=adt[:], start=True, stop=True,
                                     skip_group_check=True)

            # ---- z, e (chunk cols then self cols)
            ncol = nch + gt
            z_t = sb.tile([P, ncol, nad], F32, tag="z")
            if lw:
                as_lo = g_lo[:].rearrange("p c (h f) -> p c h f", h=nad)[:, :, :, 0]
                nc.vector.tensor_tensor(out=z_t[:, 0:lw, :], in0=as_lo,
                                        in1=zps[:, 0:lw, :], op=mybir.AluOpType.add)
            if hw:
                as_hi = g_hi[:].rearrange("p c (h f) -> p c h f", h=nad)[:, :, :, 0]
                nc.vector.tensor_tensor(out=z_t[:, lw:nch, :], in0=as_hi,
                                        in1=zps[:, lw:nch, :], op=mybir.AluOpType.add)
            nc.vector.tensor_tensor(out=z_t[:, nch:ncol, :], in0=as_own,
                                    in1=ad_own, op=mybir.AluOpType.add)
            l_t = sb.tile([P, ncol, nad], F32, tag="l")
            nc.scalar.activation(l_t[:], z_t[:],
                                 mybir.ActivationFunctionType.Prelu,
                                 alpha=NEG_SLOPE)
            e_t = sb.tile([P, ncol, nad], BF16, tag="e")
            nc.scalar.activation(e_t[:], l_t[:],
                                 mybir.ActivationFunctionType.Exp)

            # ---- denominators per tile, then r = 1/(den+eps)
            for j in range(gt):
                lo0, lcnt, hi0, hcnt = tinfo[j]
                cjs = (list(range(lo0, lo0 + lcnt))
                       + list(range(lw + hi0, lw + hi0 + hcnt)))
                for i, cj in enumerate(cjs):
                    nc.tensor.matmul(den[:, j, :], lhsT=s_v[:, cj, :],
                                     rhs=e_t[:, cj, :], start=(i == 0), stop=False,
                                     skip_group_check=True)
                nc.tensor.matmul(den[:, j, :], lhsT=ident8[:],
                                 rhs=e_t[:, nch + j, :], start=False, stop=True,
                                 skip_group_check=True)
            r_t = sb.tile([P, gt, nad], F32, tag="r")
            nc.vector.tensor_scalar_add(r_t[:], den[:], 1e-16)
            nc.vector.reciprocal(r_t[:], r_t[:])
            r16 = sb.tile([P, gt, nad], FP16, tag="r16")
            nc.vector.tensor_copy(r16[:], r_t[:])

            # ---- alpha = e * expand(r)
            for j in range(gt):
                lo0, lcnt, hi0, hcnt = tinfo[j]
                for cj in (list(range(lo0, lo0 + lcnt))
                           + list(range(lw + hi0, lw + hi0 + hcnt))):
                    nc.tensor.matmul(zps2[:, cj, :], lhsT=st_v[:, cj, :],
                                     rhs=r16[:, j, :], start=True, stop=True,
                                     skip_group_check=True)
            alpha = sb.tile([P, ncol, nad], FP16, tag="alpha")
            nc.vector.tensor_tensor(out=alpha[:, 0:nch, :], in0=e_t[:, 0:nch, :],
                                    in1=zps2, op=mybir.AluOpType.mult)
            nc.vector.tensor_tensor(out=alpha[:, nch:ncol, :],
                                    in0=e_t[:, nch:ncol, :], in1=r_t[:],
                                    op=mybir.AluOpType.mult)

            # ---- weighted messages and aggregation
            nag = row if not last else 1
            agg = psb.tile([P, gt, nag], F32, space="PSUM", tag="agg")
            # (agg and yT live in the bufs=1 pool: 2 banks each)
            if not last:
                apair = sb.tile([P, ncol, nad, 1, 2], FP16, tag="apair")
                nc.vector.tensor_copy(
                    apair[:], alpha[:].broadcast_to([P, ncol, nad, 1, 2]))
            for j in range(gt):
                lo0, lcnt, hi0, hcnt = tinfo[j]
                nch_t = lcnt + hcnt
                eg = sb.tile([P, max(nch_t, 1), nag], FP16, tag="eg")
                eg_s = sb.tile([P, nag], FP16, tag="egs")
                if not last:
                    egv = eg[:].rearrange("p c (h r t) -> p c h r t", h=H, t=2)
                    if lcnt:
                        nc.vector.tensor_tensor(
                            out=egv[:, 0:lcnt],
                            in0=g_lo[:, lo0:lo0 + lcnt, :].rearrange(
                                "p c (h r t) -> p c h r t", h=H, t=2),
                            in1=apair[:, lo0:lo0 + lcnt].broadcast_to(
                                [P, lcnt, H, 32, 2]),
                            op=mybir.AluOpType.mult)
                    if hcnt:
                        nc.vector.tensor_tensor(
                            out=egv[:, lcnt:nch_t],
                            in0=g_hi[:, hi0:hi0 + hcnt, :].rearrange(
                                "p c (h r t) -> p c h r t", h=H, t=2),
                            in1=apair[:, lw + hi0:lw + hi0 + hcnt].broadcast_to(
                                [P, hcnt, H, 32, 2]),
                            op=mybir.AluOpType.mult)
                    nc.vector.tensor_tensor(
                        out=eg_s[:].rearrange("p (h r t) -> p h r t", h=H, t=2),
                        in0=selfr[:, j, :].rearrange("p (h r t) -> p h r t",
                                                     h=H, t=2),
                        in1=apair[:, nch + j].broadcast_to([P, H, 32, 2]),
                        op=mybir.AluOpType.mult)
                else:
                    if lcnt:
                        nc.vector.tensor_tensor(
                            out=eg[:, 0:lcnt, :], in0=g_lo[:, lo0:lo0 + lcnt, 0:1],
                            in1=alpha[:, lo0:lo0 + lcnt, :], op=mybir.AluOpType.mult)
                    if hcnt:
                        nc.vector.tensor_tensor(
                            out=eg[:, lcnt:nch_t, :],
                            in0=g_hi[:, hi0:hi0 + hcnt, 0:1],
                            in1=alpha[:, lw + hi0:lw + hi0 + hcnt, :],
                            op=mybir.AluOpType.mult)
                    nc.vector.tensor_tensor(
                        out=eg_s[:], in0=selfr[:, j, 0:1],
                        in1=alpha[:, nch + j, :], op=mybir.AluOpType.mult)
                cjs = (list(range(lo0, lo0 + lcnt))
                       + list(range(lw + hi0, lw + hi0 + hcnt)))
                for i, cj in enumerate(cjs):
                    nc.tensor.matmul(agg[:, j, :], lhsT=s_v[:, cj, :],
                                     rhs=eg[:, i, :], start=(i == 0), stop=False)
                nc.tensor.matmul(agg[:, j, :], lhsT=ident8[:],
                                 rhs=eg_s[:], start=(nch_t == 0), stop=True)

            # ---- epilogue
            if last:
                for j in range(gt):
                    t = t0 + j
                    nc.vector.tensor_scalar(
                        out=obuf[:, t:t + 1], in0=agg[:, j, :],
                        scalar1=float(inv_as3), scalar2=float(b3),
                        op0=mybir.AluOpType.mult, op1=mybir.AluOpType.add)
                continue

            yT = psb.tile([P, gt, 2, P], F32, space="PSUM", tag="yT")
            for j in range(gt):
                xn = sb.tile([P, F], FP16, tag="xn")
                nc.scalar.copy(xn[:], agg[:, j, :])
                for k in range(2):
                    xT_ps = psb.tile([P, P], FP16, space="PSUM", tag="xT")
                    nc.tensor.transpose(xT_ps[:], xn[:, k * P:(k + 1) * P],
                                        identb[:])
                    xTs = sb.tile([P, P], FP16, tag="xTs")
                    nc.scalar.copy(xTs[:], xT_ps[:])
                    nc.tensor.matmul(yT[:, j, k, :], lhsT=minv_t[:, k, :],
                                     rhs=xTs[:], start=True, stop=True)
            xe_g = sb.tile([P, gt, 2, P], FP16, tag="xe")
            for k in range(2):
                p_k = sb.tile([P, gt, P], FP16, tag="pk")
                nc.scalar.activation(p_k[:], yT[:, :, k, :],
                                     mybir.ActivationFunctionType.Relu,
                                     bias=bias_t[:, k:k + 1])
                m_k = sb.tile([P, gt, P], F32, tag="mk")
                nc.scalar.activation(m_k[:], yT[:, :, k, :],
                                     mybir.ActivationFunctionType.Relu,
                                     bias=nbias_t[:, k:k + 1], scale=-1.0)
                q_k = sb.tile([P, gt, P], FP16, tag="qk")
                nc.scalar.activation(q_k[:], m_k[:],
                                     mybir.ActivationFunctionType.Exp,
                                     scale=-1.0)
                nc.vector.tensor_tensor(out=xe_g[:, :, k, :], in0=p_k[:],
                                        in1=q_k[:], op=mybir.AluOpType.add)
            trow = sb.tile([P, gt, ROW if layer == 1 else ROW3], FP16, tag="trow")
            if layer == 2:
                nc.vector.memset(trow[:], 0.0)
            for j in range(gt):
                h_ps = psb.tile([P, naug], F32, space="PSUM", tag="hps")
                for k in range(2):
                    nc.tensor.matmul(h_ps[:], lhsT=xe_g[:, j, k, :],
                                     rhs=waug_t[:, k, :], start=(k == 0),
                                     stop=(k == 1))
                if layer == 1:
                    nc.scalar.copy(trow[:, j, :], h_ps[:])
                    # overwrite as/ad slots with the exact (-1-corrected) values
                    nc.vector.tensor_tensor(
                        out=trow[:, j, :].rearrange("p (h c) -> p h c",
                                                    h=H)[:, :, 0:2],
                        in0=h_ps[:].rearrange("p (h c) -> p h c", h=H)[:, :, 0:2],
                        in1=wc_t[:].rearrange("p (h c) -> p h c", h=H),
                        op=mybir.AluOpType.subtract)
                else:
                    nc.vector.tensor_tensor(
                        out=trow[:, j, 0:2], in0=h_ps[:], in1=wc_t[:],
                        op=mybir.AluOpType.subtract)
            nc.scalar.dma_start(
                tabout[t0 * P:(t0 + gt) * P, :].rearrange("(b p) f -> p b f", p=P),
                trow[:])
        if last:
            nc.scalar.dma_start(
                outv.rearrange("(t p) o -> p t o", p=P).squeeze(-1), obuf[:])
    nc.compile()
    return pr


# --------------------------------------------------------------- the kernel

LAST_TIMES = {}


def _run(pr, in_maps, tag=None):
    if tag is not None:
        try:
            from concourse.timeline_sim import TimelineSim
            LAST_TIMES[tag] = TimelineSim(pr.nc, trace=False).simulate() / 1e9
        except Exception:
            pass
    res = bass_utils.run_bass_kernel_spmd(
        pr.nc, in_maps, core_ids=list(range(N_CORES)))
    return res.results


def _pad_rows(a, n):
    out = np.zeros((n,) + a.shape[1:], a.dtype)
    out[:len(a)] = a
    return out


def kernel(x, edge_index, W1, a_src1, a_dst1, b1, W2, a_src2, a_dst2, b2,
           W3, a_src3, a_dst3, b3):
    x = np.asarray(x, np.float32)
    ei = np.asarray(edge_index)
    src = ei[0].astype(np.int64)
    dst = ei[1].astype(np.int64)

    pos = _balance_perm(dst)
    node_of_pos = np.empty(N_NODES, np.int64)
    node_of_pos[pos] = np.arange(N_NODES)
    sch = build_schedule(src, dst, pos)

    W1 = np.asarray(W1, np.float64)
    W2 = np.asarray(W2, np.float64)
    W3 = np.asarray(W3, np.float64)
    M1 = _rot_M(np.asarray(a_src1), np.asarray(a_dst1))
    M2 = _rot_M(np.asarray(a_src2), np.asarray(a_dst2))
    BD1, BD2 = _blockdiag(M1), _blockdiag(M2)
    W1rot = (W1 @ BD1).astype(np.float32)
    W2rot = (W2 @ BD2).astype(np.float32)
    a_s3 = float(np.asarray(a_src3).reshape(-1)[0])
    a_d3 = float(np.asarray(a_dst3).reshape(-1)[0])
    W3aug = np.concatenate([W3 * a_s3, W3 * a_d3], 1).astype(np.float32)

    Minv1 = _blockdiag([np.linalg.inv(m) for m in M1])
    Minv2 = _blockdiag([np.linalg.inv(m) for m in M2])
    minv1_t = np.stack([Minv1[k * P:(k + 1) * P, k * P:(k + 1) * P]
                        for k in range(2)]).transpose(1, 0, 2)
    minv2_t = np.stack([Minv2[k * P:(k + 1) * P, k * P:(k + 1) * P]
                        for k in range(2)]).transpose(1, 0, 2)

    w2row = W2rot.sum(0).astype(np.float64)            # ones @ W2rot
    w3row = W3aug.sum(0).astype(np.float64)
    wc2 = np.tile(w2row[SLOTS01].astype(np.float32), (P, 1))
    wc3 = np.tile(w3row.astype(np.float32), (P, 1))
    w2m = w2row.copy()
    w2m[SLOTS01] = 0.0
    b1v = np.asarray(b1, np.float64)
    b2v = np.asarray(b2, np.float64)
    b1_eff = b1v
    b2_eff = b2v - (w2m @ Minv2)
    b1T = b1_eff.astype(np.float32).reshape(2, P).T.copy()
    b2T = b2_eff.astype(np.float32).reshape(2, P).T.copy()

    bf = np.float16
    consts1 = dict(waug=W2rot.astype(bf), wc=wc2, minv=minv1_t.astype(bf),
                   bias=b1T, nbias=np.ascontiguousarray(-b1T))
    consts2 = dict(waug=W3aug.astype(bf), wc=wc3, minv=minv2_t.astype(bf),
                   bias=b2T, nbias=np.ascontiguousarray(-b2T))

    xp = x[node_of_pos]                                 # permuted rows

    prA = build_launch_A()
    inA = [dict(x=_pad_rows(xp[c * NS:(c + 1) * NS], NSP), w1=W1rot)
           for c in range(N_CORES)]
    resA = _run(prA, inA, tag="A")
    tab1 = np.ascontiguousarray(
        np.concatenate([resA[c]["tab"][:NS] for c in range(N_CORES)], 0))

    prB = build_launch_agg(sch, 1)
    inB = [dict(table=tab1, mytab=_pad_rows(tab1[c * NS:(c + 1) * NS], NSP),
                blob=sch["blob"][c], **consts1) for c in range(N_CORES)]
    resB = _run(prB, inB, tag="B")
    tab2 = np.ascontiguousarray(
        np.concatenate([resB[c]["tabout"][:NS] for c in range(N_CORES)], 0))

    prC = build_launch_agg(sch, 2)
    inC = [dict(table=tab2, mytab=_pad_rows(tab2[c * NS:(c + 1) * NS], NSP),
                blob=sch["blob"][c], **consts2) for c in range(N_CORES)]
    resC = _run(prC, inC, tag="C")
    tab3 = np.ascontiguousarray(
        np.concatenate([resC[c]["tabout"][:NS] for c in range(N_CORES)], 0))

    prD = build_launch_agg(sch, 3, b3=float(np.asarray(b3).reshape(-1)[0]),
                           inv_as3=1.0 / a_s3)
    inD = [dict(table=tab3, mytab=_pad_rows(tab3[c * NS:(c + 1) * NS], NSP),
                blob=sch["blob"][c]) for c in range(N_CORES)]
    resD = _run(prD, inD, tag="D")
    outp = np.concatenate([resD[c]["outv"][:NS] for c in range(N_CORES)], 0)
    out = outp[pos]                                     # back to node order
    return np.ascontiguousarray(out.astype(np.float32))


# revision 34
# speedup vs baseline: 1.0701x; 1.0701x over previous
"""GAT (3-layer, PyG-style) on 8 Trainium2 NeuronCores via Bass/Tile.

Strategy (dst-sharded graph parallel, v2):
  - Nodes permuted for per-tile load balance, sharded 8 ways by destination;
    edges partitioned by (core, dst tile), split lo/hi by source position
    (int16 gather limit), chunked to 128 edges.
  - Per layer a node table [N, 256] bf16 holds the head-features ROTATED so
    that slots 0,1 of each head are exactly al_src / al_dst (basis matrix M
    with columns [a_src | a_dst | orthonormal complement] folded into W).
    512B rows hit the DMA descriptor cost floor; the inverse rotation is one
    extra 128-col matmul per half in the epilogue.
  - Self-loops are a constant-identity chunk fed from the core's own rows
    (no gather, no one-hot bytes).
  - idx16 | S | ST packed into one blob DMA per 4-tile group; softmax is
    pre-normalized: denominators first (4-col matmuls), 1/den expanded per
    edge via the already-loaded ST, then alpha replaces e everywhere.
  - alpha replicated into bf16 pairs so the big alpha*h multiply runs in the
    DVE 2x_1p mode; epilogue matmuls run bf16/f32r at 1 cycle/column.
"""
import numpy as np
import ml_dtypes
from contextlib import ExitStack

import concourse.bass as bass
import concourse.tile as tile
from concourse import bacc, mybir
from concourse import bass_utils
from concourse.masks import make_identity

P = 128
N_NODES = 50000
N_EDGES = 650000
NEG_SLOPE = 0.2
N_CORES = 8
NS = N_NODES // N_CORES            # 6250 nodes per shard
NT = (NS + P - 1) // P             # 49 dst tiles per core
NSP = NT * P                       # padded shard nodes (6272)
HALF = 32768                       # int16 gather limit -> low/high split
ROW = 256                          # bf16 slots per table row (512B), layers 1,2
ROW3 = 128                         # bf16 slots per layer-3 table row (256B)
F = 256                            # feature width (H*C)
H = 4
GT = 4                             # tiles per group
GMAX = 8                           # chunks per dma_gather (1024-desc ucode cap)

FP8 = mybir.dt.float8e4
BF16 = mybir.dt.bfloat16
FP16 = mybir.dt.float16
F32 = mybir.dt.float32
F32R = mybir.dt.float32r
I16 = mybir.dt.int16
U8 = mybir.dt.uint8

SLOTS01 = [h * 64 + s for h in range(H) for s in range(2)]


# ----------------------------------------------------------------- host prep

def _wrap16(idx_flat):
    """[n] int array -> [128, n//16] int16 (16-partition wrap, replicated)."""
    n = len(idx_flat)
    a = np.asarray(idx_flat, dtype=np.int16).reshape(n // 16, 16).T
    return np.tile(a, (8, 1))


def _balance_perm(dst):
    """Permute nodes so per-(core,tile) edge counts are balanced.

    Returns pos_of_node [N] (permuted global position: core*NS + local)."""
    deg = np.bincount(dst, minlength=N_NODES)
    order = np.argsort(-deg, kind="stable")
    NB = N_CORES * NT                       # bins; bin b -> (tile b//8, core b%8)
    cap = np.empty(NB, np.int64)
    for t in range(NT):
        c = 128 if t < NT - 1 else NS - 128 * (NT - 1)
        cap[t * N_CORES:(t + 1) * N_CORES] = c
    # snake deal by descending degree over non-full bins
    snake = []
    for r in range(128):
        idxs = [b for b in range(NB) if cap[b] > r]
        if r % 2:
            idxs = idxs[::-1]
        snake.extend(idxs)
    assert len(snake) == N_NODES
    fill = np.zeros(NB, np.int64)
    pos_of_node = np.empty(N_NODES, np.int64)
    for i, n in enumerate(order):
        b = snake[i]
        t, c = b // N_CORES, b % N_CORES
        pos_of_node[n] = c * NS + t * 128 + fill[b]
        fill[b] += 1
    return pos_of_node


def build_schedule(src, dst, pos_of_node):
    """Edges by (core, tile, half); per-tile chunk counts maxed over cores;
    blob = [idx16 | S | ST] per 4-tile group."""
    spos = pos_of_node[src]
    dpos = pos_of_node[dst]
    core_of = dpos // NS
    local = dpos % NS
    tile_of = local // 128
    dslot = local % 128
    is_hi = spos >= HALF

    # per (core, tile): lo and hi edge arrays (srcidx, dslot)
    per = {}
    okey = core_of * (NT * 2) + tile_of * 2 + is_hi
    order = np.argsort(okey, kind="stable")
    so, do_, ko = spos[order], dslot[order], okey[order]
    bounds = np.searchsorted(ko, np.arange(N_CORES * NT * 2 + 1))
    cnt = np.zeros((N_CORES, NT, 2), np.int64)
    for c in range(N_CORES):
        for t in range(NT):
            for hh in range(2):
                k = c * (NT * 2) + t * 2 + hh
                b0, b1 = bounds[k], bounds[k + 1]
                per[(c, t, hh)] = (so[b0:b1] - (HALF if hh else 0), do_[b0:b1])
                cnt[c, t, hh] = b1 - b0

    LCH = np.maximum(-(-cnt[:, :, 0].max(0) // 128), 0)
    HCH = np.maximum(-(-cnt[:, :, 1].max(0) // 128), 0)

    # groups of GT tiles
    groups = []
    t0 = 0
    while t0 < NT:
        gt = min(GT, NT - t0)
        groups.append((t0, gt))
        t0 += gt

    # chunk layout per group: [tile-major lo][tile-major hi]
    meta = []
    TOTCH = int(LCH.sum() + HCH.sum())
    idx_all = np.zeros((N_CORES, P, TOTCH * 8), np.int16)
    one = ml_dtypes.float8_e4m3(1.0)
    S = np.zeros((N_CORES, P, TOTCH, P), ml_dtypes.float8_e4m3)
    ST = np.zeros((N_CORES, P, TOTCH, P), ml_dtypes.float8_e4m3)
    ch = 0
    for (t0, gt) in groups:
        lw = int(LCH[t0:t0 + gt].sum())
        hw = int(HCH[t0:t0 + gt].sum())
        tinfo = []
        lo_off = 0
        hi_off = 0
        for j in range(gt):
            t = t0 + j
            tinfo.append((int(lo_off), int(LCH[t]), int(hi_off), int(HCH[t])))
            lo_off += int(LCH[t])
            hi_off += int(HCH[t])
        meta.append(dict(t0=t0, gt=gt, lw=lw, hw=hw, ch0=ch, tinfo=tinfo))
        for c in range(N_CORES):
            for j in range(gt):
                t = t0 + j
                for hh in range(2):
                    nch = int((LCH[t], HCH[t])[hh])
                    if nch == 0:
                        continue
                    base = ch + (tinfo[j][0] if hh == 0 else lw + tinfo[j][2])
                    es, ed = per[(c, t, hh)]
                    n = nch * 128
                    e_pad = np.zeros(n, np.int64)
                    e_pad[:len(es)] = es
                    if len(es):
                        k = np.arange(len(es))
                        S[c, k % P, base + k // P, ed] = one
                        ST[c, ed, base + k // P, k % P] = one
                    idx_all[c, :, base * 8:(base + nch) * 8] = _wrap16(e_pad)
        ch += lw + hw

    # blob: per group [idx (nch*16B) | S (nch*128B) | ST (nch*128B)]
    blob_parts = [[] for _ in range(N_CORES)]
    off = 0
    for m in meta:
        c0, nch = m["ch0"], m["lw"] + m["hw"]
        m["blob_off"] = off
        m["nch"] = nch
        off += nch * (16 + 128 + 128)
        for c in range(N_CORES):
            blob_parts[c].append(idx_all[c, :, c0 * 8:(c0 + nch) * 8].view(np.uint8))
            blob_parts[c].append(S[c, :, c0:c0 + nch, :].reshape(P, nch * 128).view(np.uint8))
            blob_parts[c].append(ST[c, :, c0:c0 + nch, :].reshape(P, nch * 128).view(np.uint8))
    blob = np.stack([np.concatenate(blob_parts[c], axis=1) for c in range(N_CORES)])
    return dict(meta=meta, TOTB=off, TOTCH=TOTCH, blob=np.ascontiguousarray(blob),
                LCH=LCH, HCH=HCH)


def _rot_M(a_src, a_dst):
    """Per-head invertible M with columns [a_src | a_dst | complement]."""
    Hh, C = a_src.shape
    blocks = []
    for h in range(Hh):
        pair = np.stack([a_src[h], a_dst[h]], axis=1).astype(np.float64)
        Q, _ = np.linalg.qr(pair, mode="complete")
        M = np.concatenate([pair, Q[:, 2:]], axis=1)
        blocks.append(M)
    return blocks


def _blockdiag(blocks):
    n = sum(b.shape[0] for b in blocks)
    m = sum(b.shape[1] for b in blocks)
    out = np.zeros((n, m), np.float64)
    r = c = 0
    for b in blocks:
        out[r:r + b.shape[0], c:c + b.shape[1]] = b
        r += b.shape[0]
        c += b.shape[1]
    return out


# ------------------------------------------------------------- bass builders

class Prog:
    def __init__(self):
        self.nc = bacc.Bacc("TRN2", target_bir_lowering=False, debug=False,
                            num_devices=N_CORES,
                            dynamic_dma_scratch_size=32768)
        self.in_aps = {}
        self.out_aps = {}

    def inp(self, name, shape, dt):
        ap = self.nc.dram_tensor(name, list(shape), dt, kind="ExternalInput").ap()
        self.in_aps[name] = ap
        return ap

    def out(self, name, shape, dt):
        ap = self.nc.dram_tensor(name, list(shape), dt, kind="ExternalOutput").ap()
        self.out_aps[name] = ap
        return ap


def build_launch_A():
    """x_shard @ W1rot -> table1 rows (rotated h1, fp16)."""
    pr = Prog()
    nc = pr.nc
    x = pr.inp("x", [NSP, P], FP16)
    w1 = pr.inp("w1", [P, F], FP16)
    tab = pr.out("tab", [NSP, ROW], FP16)
    with tile.TileContext(nc) as tc, ExitStack() as ctx:
        sb = ctx.enter_context(tc.tile_pool(name="sb", bufs=5))
        ps = ctx.enter_context(tc.tile_pool(name="ps", bufs=4, space="PSUM"))
        cpool = ctx.enter_context(tc.tile_pool(name="cp", bufs=1))
        ident = cpool.tile([P, P], FP16)
        make_identity(nc, ident[:])
        w1t = cpool.tile([P, F], FP16)
        nc.sync.dma_start(w1t[:], w1)
        B4 = 7
        for t0 in range(0, NT, B4):
            nb = min(B4, NT - t0)
            xt = sb.tile([P, B4, P], FP16, tag="xt")
            nc.sync.dma_start(
                xt[:, 0:nb, :],
                x[t0 * P:(t0 + nb) * P, :].rearrange("(b p) f -> p b f", p=P))
            trow = sb.tile([P, B4, ROW], FP16, tag="trow")
            for j in range(nb):
                xT_ps = ps.tile([P, P], FP16, space="PSUM", tag="xT")
                nc.tensor.transpose(xT_ps[:], xt[:, j, :], ident[:])
                xT = sb.tile([P, P], FP16, tag="xTs")
                nc.scalar.copy(xT[:], xT_ps[:])
                h_ps = ps.tile([P, F], F32, space="PSUM", tag="hps")
                nc.tensor.matmul(h_ps[:], lhsT=xT[:], rhs=w1t[:],
                                 start=True, stop=True)
                nc.vector.tensor_copy(trow[:, j, :], h_ps[:])
            nc.scalar.dma_start(
                tab[t0 * P:(t0 + nb) * P, :].rearrange("(b p) f -> p b f", p=P),
                trow[:, 0:nb, :])
    nc.compile()
    return pr


def build_launch_agg(sch, layer, b3=0.0, inv_as3=1.0):
    """layer=1: L1 agg -> table2; layer=2: L2 agg -> table3; layer=3: out."""
    pr = Prog()
    nc = pr.nc
    last = layer == 3
    row = ROW3 if last else ROW
    nad = 1 if last else H
    table = pr.inp("table", [N_NODES, row], FP16)
    mytab = pr.inp("mytab", [NSP, row], FP16)
    blob_in = pr.inp("blob", [P, sch["TOTB"]], U8)
    if layer == 1:
        naug = F
        nwc = 8
        waug = pr.inp("waug", [F, naug], FP16)
        tabout = pr.out("tabout", [NSP, ROW], FP16)
    elif layer == 2:
        naug = 2
        nwc = 2
        waug = pr.inp("waug", [F, naug], FP16)
        tabout = pr.out("tabout", [NSP, ROW3], FP16)
    else:
        outv = pr.out("outv", [NSP, 1], F32)
    if not last:
        wc = pr.inp("wc", [P, nwc], F32)       # (ones@Waug)[slots], replicated
        minv = pr.inp("minv", [P, 2, P], FP16)  # per-half blockdiag(Minv)
        bias = pr.inp("bias", [P, 2], F32)
        nbias = pr.inp("nbias", [P, 2], F32)

    with tile.TileContext(nc) as tc, ExitStack() as ctx:
        sb = ctx.enter_context(tc.tile_pool(name="sb", bufs=2))
        sbg = ctx.enter_context(tc.tile_pool(name="sbg", bufs=2))
        sbgb = ctx.enter_context(tc.tile_pool(name="sbgb", bufs=3))
        ps = ctx.enter_context(tc.tile_pool(name="ps", bufs=2, space="PSUM"))
        psb = ctx.enter_context(tc.tile_pool(name="psb", bufs=1, space="PSUM"))
        # PSUM banks: combo(2) + agg(2) + yT(2) + xT(1) + hps(1) = 8
        cpool = ctx.enter_context(tc.tile_pool(name="cp", bufs=1))
        ident8 = cpool.tile([P, P], FP8)
        make_identity(nc, ident8[:])
        if not last:
            identb = cpool.tile([P, P], FP16)
            make_identity(nc, identb[:])
            waug_t = cpool.tile([P, F // P, naug], FP16, tag="waug")
            for k in range(F // P):
                nc.sync.dma_start(waug_t[:, k, :], waug[k * P:(k + 1) * P, :])
            wc_t = cpool.tile([P, nwc], F32, tag="wc")
            nc.sync.dma_start(wc_t[:], wc)
            minv_t = cpool.tile([P, 2, P], FP16, tag="minv")
            nc.sync.dma_start(minv_t[:], minv)
            bias_t = cpool.tile([P, 2], F32, tag="bias")
            nc.sync.dma_start(bias_t[:], bias)
            nbias_t = cpool.tile([P, 2], F32, tag="nbias")
            nc.sync.dma_start(nbias_t[:], nbias)
        else:
            obuf = cpool.tile([P, NT], F32, tag="obuf")

        for m in sch["meta"]:
            t0, gt, lw, hw, nch = m["t0"], m["gt"], m["lw"], m["hw"], m["nch"]
            boff = m["blob_off"]
            tinfo = m["tinfo"]
            # ---- loads
            idx_t = sbgb.tile([P, nch * 16], U8, tag="idx")
            nc.sync.dma_start(idx_t[:], blob_in[:, boff:boff + nch * 16])
            idx_v = idx_t[:].bitcast(I16)
            blob_t = sbg.tile([P, nch * 256], U8, tag="blob")
            nc.sync.dma_start(blob_t[:],
                              blob_in[:, boff + nch * 16:boff + nch * 272])
            s_v = blob_t[:, 0:nch * 128].bitcast(FP8).rearrange(
                "p (c e) -> p c e", e=P)
            st_v = blob_t[:, nch * 128:nch * 256].bitcast(FP8).rearrange(
                "p (c e) -> p c e", e=P)
            selfr = sb.tile([P, gt, row], FP16, tag="selfr")
            nc.sync.dma_start(
                selfr[:],
                mytab[t0 * P:(t0 + gt) * P, :].rearrange("(b p) f -> p b f", p=P))
            # ---- gathers
            g_lo = g_hi = None
            if lw:
                g_lo = sbg.tile([P, lw, row], FP16, tag="glo")
                for w0 in range(0, lw, GMAX):
                    w1_ = min(w0 + GMAX, lw)
                    nc.gpsimd.dma_gather(
                        out_ap=g_lo[:, w0:w1_, :], in_ap=table,
                        idxs_ap=idx_v[:, w0 * 8:w1_ * 8],
                        num_idxs=(w1_ - w0) * P, num_idxs_reg=(w1_ - w0) * P,
                        elem_size=row)
            if hw:
                g_hi = sbg.tile([P, hw, row], FP16, tag="ghi")
                for w0 in range(0, hw, GMAX):
                    w1_ = min(w0 + GMAX, hw)
                    nc.gpsimd.dma_gather(
                        out_ap=g_hi[:, w0:w1_, :], in_ap=table[HALF:, :],
                        idxs_ap=idx_v[:, (lw + w0) * 8:(lw + w1_) * 8],
                        num_idxs=(w1_ - w0) * P, num_idxs_reg=(w1_ - w0) * P,
                        elem_size=row)

            # ---- as/ad of own nodes (slots 0,1 per head)
            if not last:
                selfr_v = selfr[:].rearrange("p b (h c) -> p b h c", h=H)
                asad = sb.tile([P, gt, H, 2], FP16, tag="asad")
                nc.vector.tensor_copy(asad[:], selfr_v[:, :, :, 0:2])
                as_own, ad_own = asad[:, :, :, 0], asad[:, :, :, 1]
            else:
                as_own, ad_own = selfr[:, :, 0:1], selfr[:, :, 1:2]

            # ---- a_dst expansion (zps) per chunk; combo also holds
            # the r-expansion (zps2) and per-tile denominators in one bank
            combo = ps.tile([P, 2 * nch + gt, nad], F32, space="PSUM", tag="combo")
            zps = combo[:, 0:nch, :]
            zps2 = combo[:, nch:2 * nch, :]
            den = combo[:, 2 * nch:2 * nch + gt, :]
            for j in range(gt):
                lo0, lcnt, hi0, hcnt = tinfo[j]
                adt = sb.tile([P, nad], FP16, tag="adt")
                nc.vector.tensor_copy(adt[:], ad_own[:, j, :])
                for cj in (list(range(lo0, lo0 + lcnt))
                           + list(range(lw + hi0, lw + hi0 + hcnt))):
                    nc.tensor.matmul(zps[:, cj, :], lhsT=st_v[:, cj, :],
                                     rhs=adt[:], start=True, stop=True,
                                     skip_group_check=True)

            # ---- z, e (chunk cols then self cols)
            ncol = nch + gt
            z_t = sb.tile([P, ncol, nad], F32, tag="z")
            if lw:
                as_lo = g_lo[:].rearrange("p c (h f) -> p c h f", h=nad)[:, :, :, 0]
                nc.vector.tensor_tensor(out=z_t[:, 0:lw, :], in0=as_lo,
                                        in1=zps[:, 0:lw, :], op=mybir.AluOpType.add)
            if hw:
                as_hi = g_hi[:].rearrange("p c (h f) -> p c h f", h=nad)[:, :, :, 0]
                nc.vector.tensor_tensor(out=z_t[:, lw:nch, :], in0=as_hi,
                                        in1=zps[:, lw:nch, :], op=mybir.AluOpType.add)
            nc.vector.tensor_tensor(out=z_t[:, nch:ncol, :], in0=as_own,
                                    in1=ad_own, op=mybir.AluOpType.add)
            l_t = sb.tile([P, ncol, nad], F32, tag="l")
            nc.scalar.activation(l_t[:], z_t[:],
                                 mybir.ActivationFunctionType.Prelu,
                                 alpha=NEG_SLOPE)
            e_t = sb.tile([P, ncol, nad], BF16, tag="e")
            nc.scalar.activation(e_t[:], l_t[:],
                                 mybir.ActivationFunctionType.Exp)

            # ---- denominators per tile, then r = 1/(den+eps)
            for j in range(gt):
                lo0, lcnt, hi0, hcnt = tinfo[j]
                cjs = (list(range(lo0, lo0 + lcnt))
                       + list(range(lw + hi0, lw + hi0 + hcnt)))
                for i, cj in enumerate(cjs):
                    nc.tensor.matmul(den[:, j, :], lhsT=s_v[:, cj, :],
                                     rhs=e_t[:, cj, :], start=(i == 0), stop=False,
                                     skip_group_check=True)
                nc.tensor.matmul(den[:, j, :], lhsT=ident8[:],
                                 rhs=e_t[:, nch + j, :], start=False, stop=True,
                                 skip_group_check=True)
            r_t = sb.tile([P, gt, nad], F32, tag="r")
            nc.vector.tensor_scalar_add(r_t[:], den[:], 1e-16)
            nc.vector.reciprocal(r_t[:], r_t[:])
            r16 = sb.tile([P, gt, nad], FP16, tag="r16")
            nc.vector.tensor_copy(r16[:], r_t[:])

            # ---- alpha = e * expand(r)
            for j in range(gt):
                lo0, lcnt, hi0, hcnt = tinfo[j]
                for cj in (list(range(lo0, lo0 + lcnt))
                           + list(range(lw + hi0, lw + hi0 + hcnt))):
                    nc.tensor.matmul(zps2[:, cj, :], lhsT=st_v[:, cj, :],
                                     rhs=r16[:, j, :], start=True, stop=True,
                                     skip_group_check=True)
            alpha = sb.tile([P, ncol, nad], FP16, tag="alpha")
            nc.vector.tensor_tensor(out=alpha[:, 0:nch, :], in0=e_t[:, 0:nch, :],
                                    in1=zps2, op=mybir.AluOpType.mult)
            nc.vector.tensor_tensor(out=alpha[:, nch:ncol, :],
                                    in0=e_t[:, nch:ncol, :], in1=r_t[:],
                                    op=mybir.AluOpType.mult)

            # ---- weighted messages and aggregation
            nag = row if not last else 1
            agg = psb.tile([P, gt, nag], F32, space="PSUM", tag="agg")
            # (agg and yT live in the bufs=1 pool: 2 banks each)
            if not last:
                apair = sb.tile([P, ncol, nad, 1, 2], FP16, tag="apair")
                nc.vector.tensor_copy(
                    apair[:], alpha[:].broadcast_to([P, ncol, nad, 1, 2]))
            for j in range(gt):
                lo0, lcnt, hi0, hcnt = tinfo[j]
                nch_t = lcnt + hcnt
                eg = sb.tile([P, max(nch_t, 1), nag], FP16, tag="eg")
                eg_s = sb.tile([P, nag], FP16, tag="egs")
                if not last:
                    egv = eg[:].rearrange("p c (h r t) -> p c h r t", h=H, t=2)
                    if lcnt:
                        nc.vector.tensor_tensor(
                            out=egv[:, 0:lcnt],
                            in0=g_lo[:, lo0:lo0 + lcnt, :].rearrange(
                                "p c (h r t) -> p c h r t", h=H, t=2),
                            in1=apair[:, lo0:lo0 + lcnt].broadcast_to(
                                [P, lcnt, H, 32, 2]),
                            op=mybir.AluOpType.mult)
                    if hcnt:
                        nc.vector.tensor_tensor(
                            out=egv[:, lcnt:nch_t],
                            in0=g_hi[:, hi0:hi0 + hcnt, :].rearrange(
                                "p c (h r t) -> p c h r t", h=H, t=2),
                            in1=apair[:, lw + hi0:lw + hi0 + hcnt].broadcast_to(
                                [P, hcnt, H, 32, 2]),
                            op=mybir.AluOpType.mult)
                    nc.vector.tensor_tensor(
                        out=eg_s[:].rearrange("p (h r t) -> p h r t", h=H, t=2),
                        in0=selfr[:, j, :].rearrange("p (h r t) -> p h r t",
                                                     h=H, t=2),
                        in1=apair[:, nch + j].broadcast_to([P, H, 32, 2]),
                        op=mybir.AluOpType.mult)
                else:
                    if lcnt:
                        nc.vector.tensor_tensor(
                            out=eg[:, 0:lcnt, :], in0=g_lo[:, lo0:lo0 + lcnt, 0:1],
                            in1=alpha[:, lo0:lo0 + lcnt, :], op=mybir.AluOpType.mult)
                    if hcnt:
                        nc.vector.tensor_tensor(
                            out=eg[:, lcnt:nch_t, :],
                            in0=g_hi[:, hi0:hi0 + hcnt, 0:1],
                            in1=alpha[:, lw + hi0:lw + hi0 + hcnt, :],
                            op=mybir.AluOpType.mult)
                    nc.vector.tensor_tensor(
                        out=eg_s[:], in0=selfr[:, j, 0:1],
                        in1=alpha[:, nch + j, :], op=mybir.AluOpType.mult)
                cjs = (list(range(lo0, lo0 + lcnt))
                       + list(range(lw + hi0, lw + hi0 + hcnt)))
                for i, cj in enumerate(cjs):
                    nc.tensor.matmul(agg[:, j, :], lhsT=s_v[:, cj, :],
                                     rhs=eg[:, i, :], start=(i == 0), stop=False)
                nc.tensor.matmul(agg[:, j, :], lhsT=ident8[:],
                                 rhs=eg_s[:], start=(nch_t == 0), stop=True)

            # ---- epilogue
            if last:
                for j in range(gt):
                    t = t0 + j
                    nc.vector.tensor_scalar(
                        out=obuf[:, t:t + 1], in0=agg[:, j, :],
                        scalar1=float(inv_as3), scalar2=float(b3),
                        op0=mybir.AluOpType.mult, op1=mybir.AluOpType.add)
                continue

            yT = psb.tile([P, gt, 2, P], F32, space="PSUM", tag="yT")
            for j in range(gt):
                xn = sb.tile([P, F], FP16, tag="xn")
                nc.scalar.copy(xn[:], agg[:, j, :])
                for k in range(2):
                    xT_ps = psb.tile([P, P], FP16, space="PSUM", tag="xT")
                    nc.tensor.transpose(xT_ps[:], xn[:, k * P:(k + 1) * P],
                                        identb[:])
                    xTs = sb.tile([P, P], FP16, tag="xTs")
                    nc.scalar.copy(xTs[:], xT_ps[:])
                    nc.tensor.matmul(yT[:, j, k, :], lhsT=minv_t[:, k, :],
                                     rhs=xTs[:], start=True, stop=True)
            xe_g = sb.tile([P, gt, 2, P], FP16, tag="xe")
            for k in range(2):
                p_k = sb.tile([P, gt, P], FP16, tag="pk")
                nc.scalar.activation(p_k[:], yT[:, :, k, :],
                                     mybir.ActivationFunctionType.Relu,
                                     bias=bias_t[:, k:k + 1])
                m_k = sb.tile([P, gt, P], F32, tag="mk")
                nc.scalar.activation(m_k[:], yT[:, :, k, :],
                                     mybir.ActivationFunctionType.Relu,
                                     bias=nbias_t[:, k:k + 1], scale=-1.0)
                q_k = sb.tile([P, gt, P], FP16, tag="qk")
                nc.scalar.activation(q_k[:], m_k[:],
                                     mybir.ActivationFunctionType.Exp,
                                     scale=-1.0)
                nc.vector.tensor_tensor(out=xe_g[:, :, k, :], in0=p_k[:],
                                        in1=q_k[:], op=mybir.AluOpType.add)
            trow = sb.tile([P, gt, ROW if layer == 1 else ROW3], FP16, tag="trow")
            if layer == 2:
                nc.vector.memset(trow[:], 0.0)
            for j in range(gt):
                h_ps = psb.tile([P, naug], F32, space="PSUM", tag="hps")
                for k in range(2):
                    nc.tensor.matmul(h_ps[:], lhsT=xe_g[:, j, k, :],
                                     rhs=waug_t[:, k, :], start=(k == 0),
                                     stop=(k == 1))
                if layer == 1:
                    nc.scalar.copy(trow[:, j, :], h_ps[:])
                    # overwrite as/ad slots with the exact (-1-corrected) values
                    nc.vector.tensor_tensor(
                        out=trow[:, j, :].rearrange("p (h c) -> p h c",
                                                    h=H)[:, :, 0:2],
                        in0=h_ps[:].rearrange("p (h c) -> p h c", h=H)[:, :, 0:2],
                        in1=wc_t[:].rearrange("p (h c) -> p h c", h=H),
                        op=mybir.AluOpType.subtract)
                else:
                    nc.vector.tensor_tensor(
                        out=trow[:, j, 0:2], in0=h_ps[:], in1=wc_t[:],
                        op=mybir.AluOpType.subtract)
            nc.scalar.dma_start(
                tabout[t0 * P:(t0 + gt) * P, :].rearrange("(b p) f -> p b f", p=P),
                trow[:])
        if last:
            nc.scalar.dma_start(
                outv.rearrange("(t p) o -> p t o", p=P).squeeze(-1), obuf[:])
    nc.compile()
    return pr


# --------------------------------------------------------------- the kernel

LAST_TIMES = {}


def _run(pr, in_maps, tag=None):
    if tag is not None:
        try:
            from concourse.timeline_sim import TimelineSim
            LAST_TIMES[tag] = TimelineSim(pr.nc, trace=False).simulate() / 1e9
        except Exception:
            pass
    res = bass_utils.run_bass_kernel_spmd(
        pr.nc, in_maps, core_ids=list(range(N_CORES)))
    return res.results


def _pad_rows(a, n):
    out = np.zeros((n,) + a.shape[1:], a.dtype)
    out[:len(a)] = a
    return out


def kernel(x, edge_index, W1, a_src1, a_dst1, b1, W2, a_src2, a_dst2, b2,
           W3, a_src3, a_dst3, b3):
    x = np.asarray(x, np.float32)
    ei = np.asarray(edge_index)
    src = ei[0].astype(np.int64)
    dst = ei[1].astype(np.int64)

    pos = _balance_perm(dst)
    node_of_pos = np.empty(N_NODES, np.int64)
    node_of_pos[pos] = np.arange(N_NODES)
    sch = build_schedule(src, dst, pos)

    W1 = np.asarray(W1, np.float64)
    W2 = np.asarray(W2, np.float64)
    W3 = np.asarray(W3, np.float64)
    M1 = _rot_M(np.asarray(a_src1), np.asarray(a_dst1))
    M2 = _rot_M(np.asarray(a_src2), np.asarray(a_dst2))
    BD1, BD2 = _blockdiag(M1), _blockdiag(M2)
    W1rot = (W1 @ BD1).astype(np.float32)
    W2rot = (W2 @ BD2).astype(np.float32)
    a_s3 = float(np.asarray(a_src3).reshape(-1)[0])
    a_d3 = float(np.asarray(a_dst3).reshape(-1)[0])
    W3aug = np.concatenate([W3 * a_s3, W3 * a_d3], 1).astype(np.float32)

    Minv1 = _blockdiag([np.linalg.inv(m) for m in M1])
    Minv2 = _blockdiag([np.linalg.inv(m) for m in M2])
    minv1_t = np.stack([Minv1[k * P:(k + 1) * P, k * P:(k + 1) * P]
                        for k in range(2)]).transpose(1, 0, 2)
    minv2_t = np.stack([Minv2[k * P:(k + 1) * P, k * P:(k + 1) * P]
                        for k in range(2)]).transpose(1, 0, 2)

    w2row = W2rot.sum(0).astype(np.float64)            # ones @ W2rot
    w3row = W3aug.sum(0).astype(np.float64)
    wc2 = np.tile(w2row[SLOTS01].astype(np.float32), (P, 1))
    wc3 = np.tile(w3row.astype(np.float32), (P, 1))
    w2m = w2row.copy()
    w2m[SLOTS01] = 0.0
    b1v = np.asarray(b1, np.float64)
    b2v = np.asarray(b2, np.float64)
    b1_eff = b1v
    b2_eff = b2v - (w2m @ Minv2)
    b1T = b1_eff.astype(np.float32).reshape(2, P).T.copy()
    b2T = b2_eff.astype(np.float32).reshape(2, P).T.copy()

    bf = np.float16
    consts1 = dict(waug=W2rot.astype(bf), wc=wc2, minv=minv1_t.astype(bf),
                   bias=b1T, nbias=np.ascontiguousarray(-b1T))
    consts2 = dict(waug=W3aug.astype(bf), wc=wc3, minv=minv2_t.astype(bf),
                   bias=b2T, nbias=np.ascontiguousarray(-b2T))

    xp = x[node_of_pos]                                 # permuted rows

    prA = build_launch_A()
    xp16 = xp.astype(np.float16)
    W1rot16 = W1rot.astype(np.float16)
    inA = [dict(x=_pad_rows(xp16[c * NS:(c + 1) * NS], NSP), w1=W1rot16)
           for c in range(N_CORES)]
    resA = _run(prA, inA, tag="A")
    tab1 = np.ascontiguousarray(
        np.concatenate([resA[c]["tab"][:NS] for c in range(N_CORES)], 0))

    prB = build_launch_agg(sch, 1)
    inB = [dict(table=tab1, mytab=_pad_rows(tab1[c * NS:(c + 1) * NS], NSP),
                blob=sch["blob"][c], **consts1) for c in range(N_CORES)]
    resB = _run(prB, inB, tag="B")
    tab2 = np.ascontiguousarray(
        np.concatenate([resB[c]["tabout"][:NS] for c in range(N_CORES)], 0))

    prC = build_launch_agg(sch, 2)
    inC = [dict(table=tab2, mytab=_pad_rows(tab2[c * NS:(c + 1) * NS], NSP),
                blob=sch["blob"][c], **consts2) for c in range(N_CORES)]
    resC = _run(prC, inC, tag="C")
    tab3 = np.ascontiguousarray(
        np.concatenate([resC[c]["tabout"][:NS] for c in range(N_CORES)], 0))

    prD = build_launch_agg(sch, 3, b3=float(np.asarray(b3).reshape(-1)[0]),
                           inv_as3=1.0 / a_s3)
    inD = [dict(table=tab3, mytab=_pad_rows(tab3[c * NS:(c + 1) * NS], NSP),
                blob=sch["blob"][c]) for c in range(N_CORES)]
    resD = _run(prD, inD, tag="D")
    outp = np.concatenate([resD[c]["outv"][:NS] for c in range(N_CORES)], 0)
    out = outp[pos]                                     # back to node order
    return np.ascontiguousarray(out.astype(np.float32))


# revision 39
# speedup vs baseline: 1.2362x; 1.1551x over previous
"""GAT (3-layer, PyG-style) on 8 Trainium2 NeuronCores via Bass/Tile.

Strategy (dst-sharded graph parallel, v2):
  - Nodes permuted for per-tile load balance, sharded 8 ways by destination;
    edges partitioned by (core, dst tile), split lo/hi by source position
    (int16 gather limit), chunked to 128 edges.
  - Per layer a node table [N, 256] bf16 holds the head-features ROTATED so
    that slots 0,1 of each head are exactly al_src / al_dst (basis matrix M
    with columns [a_src | a_dst | orthonormal complement] folded into W).
    512B rows hit the DMA descriptor cost floor; the inverse rotation is one
    extra 128-col matmul per half in the epilogue.
  - Self-loops are a constant-identity chunk fed from the core's own rows
    (no gather, no one-hot bytes).
  - idx16 | S | ST packed into one blob DMA per 4-tile group; softmax is
    pre-normalized: denominators first (4-col matmuls), 1/den expanded per
    edge via the already-loaded ST, then alpha replaces e everywhere.
  - alpha replicated into bf16 pairs so the big alpha*h multiply runs in the
    DVE 2x_1p mode; epilogue matmuls run bf16/f32r at 1 cycle/column.
"""
import numpy as np
import ml_dtypes
from contextlib import ExitStack

import concourse.bass as bass
import concourse.tile as tile
from concourse import bacc, mybir
from concourse import bass_utils
from concourse.masks import make_identity

P = 128
N_NODES = 50000
N_EDGES = 650000
NEG_SLOPE = 0.2
N_CORES = 8
NS = N_NODES // N_CORES            # 6250 nodes per shard
NT = (NS + P - 1) // P             # 49 dst tiles per core
NSP = NT * P                       # padded shard nodes (6272)
HALF = 32768                       # int16 gather limit -> low/high split
ROW = 256                          # bf16 slots per table row (512B), layers 1,2
ROW3 = 128                         # bf16 slots per layer-3 table row (256B)
F = 256                            # feature width (H*C)
H = 4
GT = 4                             # tiles per group
GMAX = 8                           # chunks per dma_gather (1024-desc ucode cap)

FP8 = mybir.dt.float8e4
BF16 = mybir.dt.bfloat16
FP16 = mybir.dt.float16
F32 = mybir.dt.float32
F32R = mybir.dt.float32r
I16 = mybir.dt.int16
U8 = mybir.dt.uint8

SLOTS01 = [h * 64 + s for h in range(H) for s in range(2)]


# ----------------------------------------------------------------- host prep

def _wrap16(idx_flat):
    """[n] int array -> [128, n//16] int16 (16-partition wrap, replicated)."""
    n = len(idx_flat)
    a = np.asarray(idx_flat, dtype=np.int16).reshape(n // 16, 16).T
    return np.tile(a, (8, 1))


def _balance_perm(dst):
    """Permute nodes so per-(core,tile) edge counts are balanced.

    Returns pos_of_node [N] (permuted global position: core*NS + local)."""
    deg = np.bincount(dst, minlength=N_NODES)
    order = np.argsort(-deg, kind="stable")
    NB = N_CORES * NT                       # bins; bin b -> (tile b//8, core b%8)
    cap = np.empty(NB, np.int64)
    for t in range(NT):
        c = 128 if t < NT - 1 else NS - 128 * (NT - 1)
        cap[t * N_CORES:(t + 1) * N_CORES] = c
    # snake deal by descending degree over non-full bins
    snake = []
    for r in range(128):
        idxs = [b for b in range(NB) if cap[b] > r]
        if r % 2:
            idxs = idxs[::-1]
        snake.extend(idxs)
    assert len(snake) == N_NODES
    fill = np.zeros(NB, np.int64)
    pos_of_node = np.empty(N_NODES, np.int64)
    for i, n in enumerate(order):
        b = snake[i]
        t, c = b // N_CORES, b % N_CORES
        pos_of_node[n] = c * NS + t * 128 + fill[b]
        fill[b] += 1
    return pos_of_node


def build_schedule(src, dst, pos_of_node):
    """Edges by (core, tile, half); per-tile chunk counts maxed over cores;
    blob = [idx16 | S | ST] per 4-tile group."""
    E = len(src)
    spos = pos_of_node[src]
    dpos = pos_of_node[dst]
    core_of = dpos // NS
    local = dpos % NS
    tile_of = local // 128
    dslot = local % 128
    is_hi = spos >= HALF
    s_core = spos // NS
    s_local = spos % NS
    s_tile = s_local // 128
    s_slot = s_local % 128

    # per (core, tile): lo and hi edge arrays (srcidx, dslot)
    per = {}
    okey = core_of * (NT * 2) + tile_of * 2 + is_hi
    order = np.argsort(okey, kind="stable")
    so, do_, ko = spos[order], dslot[order], okey[order]
    bounds = np.searchsorted(ko, np.arange(N_CORES * NT * 2 + 1))
    cnt = np.zeros((N_CORES, NT, 2), np.int64)
    for c in range(N_CORES):
        for t in range(NT):
            for hh in range(2):
                k = c * (NT * 2) + t * 2 + hh
                b0, b1 = bounds[k], bounds[k + 1]
                per[(c, t, hh)] = (so[b0:b1] - (HALF if hh else 0), do_[b0:b1],
                                   order[b0:b1])
                cnt[c, t, hh] = b1 - b0

    # out-edge schedule (for the layer-3 push): per (src core, src tile)
    okey_o = s_core * NT + s_tile
    order_o = np.argsort(okey_o, kind="stable")
    bounds_o = np.searchsorted(okey_o[order_o], np.arange(N_CORES * NT + 1))
    cnt_o = (bounds_o[1:] - bounds_o[:-1]).reshape(N_CORES, NT)
    OCH = -(-cnt_o.max(0) // 128)
    oc0 = np.concatenate([[0], np.cumsum(OCH)])
    OTOT = int(oc0[-1])

    LCH = np.maximum(-(-cnt[:, :, 0].max(0) // 128), 0)
    HCH = np.maximum(-(-cnt[:, :, 1].max(0) // 128), 0)

    # groups of GT tiles
    groups = []
    t0 = 0
    while t0 < NT:
        gt = min(GT, NT - t0)
        groups.append((t0, gt))
        t0 += gt

    # chunk layout per group: [tile-major lo][tile-major hi]
    meta = []
    TOTCH = int(LCH.sum() + HCH.sum())
    dch = np.zeros(E, np.int64)
    dsl = np.zeros(E, np.int64)
    idx_all = np.zeros((N_CORES, P, TOTCH * 8), np.int16)
    one = ml_dtypes.float8_e4m3(1.0)
    S = np.zeros((N_CORES, P, TOTCH, P), ml_dtypes.float8_e4m3)
    ST = np.zeros((N_CORES, P, TOTCH, P), ml_dtypes.float8_e4m3)
    ch = 0
    for (t0, gt) in groups:
        lw = int(LCH[t0:t0 + gt].sum())
        hw = int(HCH[t0:t0 + gt].sum())
        tinfo = []
        lo_off = 0
        hi_off = 0
        for j in range(gt):
            t = t0 + j
            tinfo.append((int(lo_off), int(LCH[t]), int(hi_off), int(HCH[t])))
            lo_off += int(LCH[t])
            hi_off += int(HCH[t])
        meta.append(dict(t0=t0, gt=gt, lw=lw, hw=hw, ch0=ch, tinfo=tinfo))
        for c in range(N_CORES):
            for j in range(gt):
                t = t0 + j
                for hh in range(2):
                    nch = int((LCH[t], HCH[t])[hh])
                    if nch == 0:
                        continue
                    base = ch + (tinfo[j][0] if hh == 0 else lw + tinfo[j][2])
                    es, ed, eo = per[(c, t, hh)]
                    n = nch * 128
                    e_pad = np.zeros(n, np.int64)
                    e_pad[:len(es)] = es
                    if len(es):
                        k = np.arange(len(es))
                        S[c, k % P, base + k // P, ed] = one
                        ST[c, ed, base + k // P, k % P] = one
                        dch[eo] = base + k // P
                        dsl[eo] = k % P
                    idx_all[c, :, base * 8:(base + nch) * 8] = _wrap16(e_pad)
        ch += lw + hw

    # source-side one-hot (replication of own-node values to out-edge slots)
    Ssrc = np.zeros((N_CORES, P, OTOT, P), ml_dtypes.float8_e4m3)
    opos = np.zeros(E, np.int64)
    for c in range(N_CORES):
        for t in range(NT):
            b0, b1 = bounds_o[c * NT + t], bounds_o[c * NT + t + 1]
            eo = order_o[b0:b1]
            k = np.arange(len(eo))
            Ssrc[c, s_slot[eo], oc0[t] + k // P, k % P] = one
            opos[eo] = (oc0[t] + k // P) * P + k % P
    ssrc_blob = np.ascontiguousarray(
        Ssrc.reshape(N_CORES, P, OTOT * P).view(np.uint8))

    # blob: per group [idx (nch*16B) | S (nch*128B) | ST (nch*128B)]
    blob_parts = [[] for _ in range(N_CORES)]
    off = 0
    for m in meta:
        c0, nch = m["ch0"], m["lw"] + m["hw"]
        m["blob_off"] = off
        m["nch"] = nch
        off += nch * (16 + 128 + 128)
        for c in range(N_CORES):
            blob_parts[c].append(idx_all[c, :, c0 * 8:(c0 + nch) * 8].view(np.uint8))
            blob_parts[c].append(S[c, :, c0:c0 + nch, :].reshape(P, nch * 128).view(np.uint8))
            blob_parts[c].append(ST[c, :, c0:c0 + nch, :].reshape(P, nch * 128).view(np.uint8))
    blob = np.stack([np.concatenate(blob_parts[c], axis=1) for c in range(N_CORES)])
    return dict(meta=meta, TOTB=off, TOTCH=TOTCH, blob=np.ascontiguousarray(blob),
                LCH=LCH, HCH=HCH, OCH=OCH, oc0=oc0, OTOT=OTOT,
                ssrc=ssrc_blob, opos=opos, dch=dch, dsl=dsl,
                s_core=s_core, d_core=core_of)


def _rot_M(a_src, a_dst):
    """Per-head invertible M with columns [a_src | a_dst | complement]."""
    Hh, C = a_src.shape
    blocks = []
    for h in range(Hh):
        pair = np.stack([a_src[h], a_dst[h]], axis=1).astype(np.float64)
        Q, _ = np.linalg.qr(pair, mode="complete")
        M = np.concatenate([pair, Q[:, 2:]], axis=1)
        blocks.append(M)
    return blocks


def _blockdiag(blocks):
    n = sum(b.shape[0] for b in blocks)
    m = sum(b.shape[1] for b in blocks)
    out = np.zeros((n, m), np.float64)
    r = c = 0
    for b in blocks:
        out[r:r + b.shape[0], c:c + b.shape[1]] = b
        r += b.shape[0]
        c += b.shape[1]
    return out


# ------------------------------------------------------------- bass builders

class Prog:
    def __init__(self):
        self.nc = bacc.Bacc("TRN2", target_bir_lowering=False, debug=False,
                            num_devices=N_CORES,
                            dynamic_dma_scratch_size=32768)
        self.in_aps = {}
        self.out_aps = {}

    def inp(self, name, shape, dt):
        ap = self.nc.dram_tensor(name, list(shape), dt, kind="ExternalInput").ap()
        self.in_aps[name] = ap
        return ap

    def out(self, name, shape, dt):
        ap = self.nc.dram_tensor(name, list(shape), dt, kind="ExternalOutput").ap()
        self.out_aps[name] = ap
        return ap


def build_launch_A():
    """x_shard @ W1rot -> table1 rows (rotated h1, fp16)."""
    pr = Prog()
    nc = pr.nc
    x = pr.inp("x", [NSP, P], FP16)
    w1 = pr.inp("w1", [P, F], FP16)
    tab = pr.out("tab", [NSP, ROW], FP16)
    with tile.TileContext(nc) as tc, ExitStack() as ctx:
        sb = ctx.enter_context(tc.tile_pool(name="sb", bufs=5))
        ps = ctx.enter_context(tc.tile_pool(name="ps", bufs=4, space="PSUM"))
        cpool = ctx.enter_context(tc.tile_pool(name="cp", bufs=1))
        ident = cpool.tile([P, P], FP16)
        make_identity(nc, ident[:])
        w1t = cpool.tile([P, F], FP16)
        nc.sync.dma_start(w1t[:], w1)
        B4 = 7
        for t0 in range(0, NT, B4):
            nb = min(B4, NT - t0)
            xt = sb.tile([P, B4, P], FP16, tag="xt")
            nc.sync.dma_start(
                xt[:, 0:nb, :],
                x[t0 * P:(t0 + nb) * P, :].rearrange("(b p) f -> p b f", p=P))
            trow = sb.tile([P, B4, ROW], FP16, tag="trow")
            for j in range(nb):
                xT_ps = ps.tile([P, P], FP16, space="PSUM", tag="xT")
                nc.tensor.transpose(xT_ps[:], xt[:, j, :], ident[:])
                xT = sb.tile([P, P], FP16, tag="xTs")
                nc.scalar.copy(xT[:], xT_ps[:])
                h_ps = ps.tile([P, F], F32, space="PSUM", tag="hps")
                nc.tensor.matmul(h_ps[:], lhsT=xT[:], rhs=w1t[:],
                                 start=True, stop=True)
                nc.vector.tensor_copy(trow[:, j, :], h_ps[:])
            nc.scalar.dma_start(
                tab[t0 * P:(t0 + nb) * P, :].rearrange("(b p) f -> p b f", p=P),
                trow[:, 0:nb, :])
    nc.compile()
    return pr


def build_launch_agg(sch, layer, b3=0.0, inv_as3=1.0):
    """layer=1: L1 agg -> table2; layer=2: L2 agg -> table3; layer=3: out."""
    pr = Prog()
    nc = pr.nc
    last = layer == 3
    row = ROW3 if last else ROW
    nad = 1 if last else H
    table = pr.inp("table", [N_NODES, row], FP16)
    mytab = pr.inp("mytab", [NSP, row], FP16)
    blob_in = pr.inp("blob", [P, sch["TOTB"]], U8)
    if layer == 1:
        naug = F
        nwc = 8
        waug = pr.inp("waug", [F, naug], FP16)
        tabout = pr.out("tabout", [NSP, ROW], FP16)
    elif layer == 2:
        naug = 2
        nwc = 2
        waug = pr.inp("waug", [F, naug], FP16)
        tabout = pr.out("tabout", [NSP, ROW3], FP16)
        ssrc_in = pr.inp("ssrc", [P, sch["OTOT"] * P], U8)
        eout = pr.out("eout", [P, sch["OTOT"] * 2], FP16)
        OCH, oc0 = sch["OCH"], sch["oc0"]
    else:
        outv = pr.out("outv", [NSP, 1], F32)
        edata_in = pr.inp("edata", [P, sch["TOTCH"]], FP16)
    if not last:
        wc = pr.inp("wc", [P, nwc], F32)       # (ones@Waug)[slots], replicated
        minv = pr.inp("minv", [P, 2, P], FP16)  # per-half blockdiag(Minv)
        bias = pr.inp("bias", [P, 2], F32)
        nbias = pr.inp("nbias", [P, 2], F32)

    with tile.TileContext(nc) as tc, ExitStack() as ctx:
        sb = ctx.enter_context(tc.tile_pool(name="sb", bufs=2))
        sbg = ctx.enter_context(tc.tile_pool(name="sbg", bufs=2))
        sbgb = ctx.enter_context(tc.tile_pool(name="sbgb", bufs=3))
        ps = ctx.enter_context(tc.tile_pool(name="ps", bufs=2, space="PSUM"))
        ps1 = ctx.enter_context(tc.tile_pool(name="ps1", bufs=1, space="PSUM"))
        psb = ctx.enter_context(tc.tile_pool(name="psb", bufs=1, space="PSUM"))
        # PSUM banks (8 total):
        #   layer1: combo(ps,2) + xT/hps(psb,2) + agg/yT(psb,4)
        #   layer2: combo(ps1,1) + hps+eps(ps,2) + xT/agg/yT(psb,5)
        #   layer3: combo(ps,2) + agg(psb,1)
        cpool = ctx.enter_context(tc.tile_pool(name="cp", bufs=1))
        ident8 = cpool.tile([P, P], FP8)
        make_identity(nc, ident8[:])
        if not last:
            identb = cpool.tile([P, P], FP16)
            make_identity(nc, identb[:])
            waug_t = cpool.tile([P, F // P, naug], FP16, tag="waug")
            for k in range(F // P):
                nc.sync.dma_start(waug_t[:, k, :], waug[k * P:(k + 1) * P, :])
            wc_t = cpool.tile([P, nwc], F32, tag="wc")
            nc.sync.dma_start(wc_t[:], wc)
            minv_t = cpool.tile([P, 2, P], FP16, tag="minv")
            nc.sync.dma_start(minv_t[:], minv)
            bias_t = cpool.tile([P, 2], F32, tag="bias")
            nc.sync.dma_start(bias_t[:], bias)
            nbias_t = cpool.tile([P, 2], F32, tag="nbias")
            nc.sync.dma_start(nbias_t[:], nbias)
        else:
            obuf = cpool.tile([P, NT], F32, tag="obuf")

        for m in sch["meta"]:
            t0, gt, lw, hw, nch = m["t0"], m["gt"], m["lw"], m["hw"], m["nch"]
            boff = m["blob_off"]
            tinfo = m["tinfo"]
            ch0g = m["ch0"]
            if layer == 2:
                go0, go1 = int(oc0[t0]), int(oc0[t0 + gt])
                ochg = go1 - go0
                ssrc_t = sbg.tile([P, ochg * P], U8, tag="ssrc")
                nc.sync.dma_start(ssrc_t[:],
                                  ssrc_in[:, go0 * P:go1 * P])
                ssrc_v = ssrc_t[:].bitcast(FP8).rearrange("p (c e) -> p c e", e=P)
                eout_sb = sb.tile([P, ochg, 2], FP16, tag="eout")
            # ---- loads
            idx_t = sbgb.tile([P, nch * 16], U8, tag="idx")
            nc.sync.dma_start(idx_t[:], blob_in[:, boff:boff + nch * 16])
            idx_v = idx_t[:].bitcast(I16)
            blob_t = sbg.tile([P, nch * 256], U8, tag="blob")
            nc.sync.dma_start(blob_t[:],
                              blob_in[:, boff + nch * 16:boff + nch * 272])
            s_v = blob_t[:, 0:nch * 128].bitcast(FP8).rearrange(
                "p (c e) -> p c e", e=P)
            st_v = blob_t[:, nch * 128:nch * 256].bitcast(FP8).rearrange(
                "p (c e) -> p c e", e=P)
            selfr = sb.tile([P, gt, row], FP16, tag="selfr")
            nc.sync.dma_start(
                selfr[:],
                mytab[t0 * P:(t0 + gt) * P, :].rearrange("(b p) f -> p b f", p=P))
            # ---- gathers (layers 1,2) or pushed per-edge values (layer 3)
            g_lo = g_hi = None
            if last:
                ed_t = sbg.tile([P, nch], FP16, tag="ed")
                nc.sync.dma_start(ed_t[:], edata_in[:, ch0g:ch0g + nch])
                ed_v = ed_t[:].rearrange("p (c o) -> p c o", o=1)
            if lw and not last:
                g_lo = sbg.tile([P, lw, row], FP16, tag="glo")
                for w0 in range(0, lw, GMAX):
                    w1_ = min(w0 + GMAX, lw)
                    nc.gpsimd.dma_gather(
                        out_ap=g_lo[:, w0:w1_, :], in_ap=table,
                        idxs_ap=idx_v[:, w0 * 8:w1_ * 8],
                        num_idxs=(w1_ - w0) * P, num_idxs_reg=(w1_ - w0) * P,
                        elem_size=row)
            if hw and not last:
                g_hi = sbg.tile([P, hw, row], FP16, tag="ghi")
                for w0 in range(0, hw, GMAX):
                    w1_ = min(w0 + GMAX, hw)
                    nc.gpsimd.dma_gather(
                        out_ap=g_hi[:, w0:w1_, :], in_ap=table[HALF:, :],
                        idxs_ap=idx_v[:, (lw + w0) * 8:(lw + w1_) * 8],
                        num_idxs=(w1_ - w0) * P, num_idxs_reg=(w1_ - w0) * P,
                        elem_size=row)

            # ---- as/ad of own nodes (slots 0,1 per head)
            if not last:
                selfr_v = selfr[:].rearrange("p b (h c) -> p b h c", h=H)
                asad = sb.tile([P, gt, H, 2], FP16, tag="asad")
                nc.vector.tensor_copy(asad[:], selfr_v[:, :, :, 0:2])
                as_own, ad_own = asad[:, :, :, 0], asad[:, :, :, 1]
            else:
                as_own, ad_own = selfr[:, :, 0:1], selfr[:, :, 1:2]

            # ---- a_dst expansion (zps) per chunk; combo also holds
            # the r-expansion (zps2) and per-tile denominators in one bank
            cpoolp = ps1 if layer == 2 else ps
            combo = cpoolp.tile([P, 2 * nch + gt, nad], F32, space="PSUM",
                                tag="combo")
            zps = combo[:, 0:nch, :]
            zps2 = combo[:, nch:2 * nch, :]
            den = combo[:, 2 * nch:2 * nch + gt, :]
            for j in range(gt):
                lo0, lcnt, hi0, hcnt = tinfo[j]
                adt = sb.tile([P, nad], FP16, tag="adt")
                nc.vector.tensor_copy(adt[:], ad_own[:, j, :])
                for cj in (list(range(lo0, lo0 + lcnt))
                           + list(range(lw + hi0, lw + hi0 + hcnt))):
                    nc.tensor.matmul(zps[:, cj, :], lhsT=st_v[:, cj, :],
                                     rhs=adt[:], start=True, stop=True,
                                     skip_group_check=True)

            # ---- z, e (chunk cols then self cols)
            ncol = nch + gt
            z_t = sb.tile([P, ncol, nad], F32, tag="z")
            if last:
                nc.vector.tensor_tensor(out=z_t[:, 0:nch, :], in0=ed_v,
                                        in1=zps[:], op=mybir.AluOpType.add)
            else:
                if lw:
                    as_lo = g_lo[:].rearrange("p c (h f) -> p c h f",
                                              h=nad)[:, :, :, 0]
                    nc.vector.tensor_tensor(out=z_t[:, 0:lw, :], in0=as_lo,
                                            in1=zps[:, 0:lw, :],
                                            op=mybir.AluOpType.add)
                if hw:
                    as_hi = g_hi[:].rearrange("p c (h f) -> p c h f",
                                              h=nad)[:, :, :, 0]
                    nc.vector.tensor_tensor(out=z_t[:, lw:nch, :], in0=as_hi,
                                            in1=zps[:, lw:nch, :],
                                            op=mybir.AluOpType.add)
            nc.vector.tensor_tensor(out=z_t[:, nch:ncol, :], in0=as_own,
                                    in1=ad_own, op=mybir.AluOpType.add)
            l_t = sb.tile([P, ncol, nad], F32, tag="l")
            nc.scalar.activation(l_t[:], z_t[:],
                                 mybir.ActivationFunctionType.Prelu,
                                 alpha=NEG_SLOPE)
            e_t = sb.tile([P, ncol, nad], BF16, tag="e")
            nc.scalar.activation(e_t[:], l_t[:],
                                 mybir.ActivationFunctionType.Exp)

            # ---- denominators per tile, then r = 1/(den+eps)
            for j in range(gt):
                lo0, lcnt, hi0, hcnt = tinfo[j]
                cjs = (list(range(lo0, lo0 + lcnt))
                       + list(range(lw + hi0, lw + hi0 + hcnt)))
                for i, cj in enumerate(cjs):
                    nc.tensor.matmul(den[:, j, :], lhsT=s_v[:, cj, :],
                                     rhs=e_t[:, cj, :], start=(i == 0), stop=False,
                                     skip_group_check=True)
                nc.tensor.matmul(den[:, j, :], lhsT=ident8[:],
                                 rhs=e_t[:, nch + j, :], start=False, stop=True,
                                 skip_group_check=True)
            r_t = sb.tile([P, gt, nad], F32, tag="r")
            nc.vector.tensor_scalar_add(r_t[:], den[:], 1e-16)
            nc.vector.reciprocal(r_t[:], r_t[:])
            r16 = sb.tile([P, gt, nad], FP16, tag="r16")
            nc.vector.tensor_copy(r16[:], r_t[:])

            # ---- alpha = e * expand(r)
            for j in range(gt):
                lo0, lcnt, hi0, hcnt = tinfo[j]
                for cj in (list(range(lo0, lo0 + lcnt))
                           + list(range(lw + hi0, lw + hi0 + hcnt))):
                    nc.tensor.matmul(zps2[:, cj, :], lhsT=st_v[:, cj, :],
                                     rhs=r16[:, j, :], start=True, stop=True,
                                     skip_group_check=True)
            alpha = sb.tile([P, ncol, nad], FP16, tag="alpha")
            nc.vector.tensor_tensor(out=alpha[:, 0:nch, :], in0=e_t[:, 0:nch, :],
                                    in1=zps2, op=mybir.AluOpType.mult)
            nc.vector.tensor_tensor(out=alpha[:, nch:ncol, :],
                                    in0=e_t[:, nch:ncol, :], in1=r_t[:],
                                    op=mybir.AluOpType.mult)

            # ---- weighted messages and aggregation
            nag = row if not last else 1
            agg = psb.tile([P, gt, nag], F32, space="PSUM", tag="agg")
            # (agg and yT live in the bufs=1 pool: 2 banks each)
            if not last:
                apair = sb.tile([P, ncol, nad, 1, 2], FP16, tag="apair")
                nc.vector.tensor_copy(
                    apair[:], alpha[:].broadcast_to([P, ncol, nad, 1, 2]))
            for j in range(gt):
                lo0, lcnt, hi0, hcnt = tinfo[j]
                nch_t = lcnt + hcnt
                eg = sb.tile([P, max(nch_t, 1), nag], FP16, tag="eg")
                eg_s = sb.tile([P, nag], FP16, tag="egs")
                if not last:
                    egv = eg[:].rearrange("p c (h r t) -> p c h r t", h=H, t=2)
                    if lcnt:
                        nc.vector.tensor_tensor(
                            out=egv[:, 0:lcnt],
                            in0=g_lo[:, lo0:lo0 + lcnt, :].rearrange(
                                "p c (h r t) -> p c h r t", h=H, t=2),
                            in1=apair[:, lo0:lo0 + lcnt].broadcast_to(
                                [P, lcnt, H, 32, 2]),
                            op=mybir.AluOpType.mult)
                    if hcnt:
                        nc.vector.tensor_tensor(
                            out=egv[:, lcnt:nch_t],
                            in0=g_hi[:, hi0:hi0 + hcnt, :].rearrange(
                                "p c (h r t) -> p c h r t", h=H, t=2),
                            in1=apair[:, lw + hi0:lw + hi0 + hcnt].broadcast_to(
                                [P, hcnt, H, 32, 2]),
                            op=mybir.AluOpType.mult)
                    nc.vector.tensor_tensor(
                        out=eg_s[:].rearrange("p (h r t) -> p h r t", h=H, t=2),
                        in0=selfr[:, j, :].rearrange("p (h r t) -> p h r t",
                                                     h=H, t=2),
                        in1=apair[:, nch + j].broadcast_to([P, H, 32, 2]),
                        op=mybir.AluOpType.mult)
                else:
                    if lcnt:
                        nc.vector.tensor_tensor(
                            out=eg[:, 0:lcnt, :], in0=ed_v[:, lo0:lo0 + lcnt, :],
                            in1=alpha[:, lo0:lo0 + lcnt, :], op=mybir.AluOpType.mult)
                    if hcnt:
                        nc.vector.tensor_tensor(
                            out=eg[:, lcnt:nch_t, :],
                            in0=ed_v[:, lw + hi0:lw + hi0 + hcnt, :],
                            in1=alpha[:, lw + hi0:lw + hi0 + hcnt, :],
                            op=mybir.AluOpType.mult)
                    nc.vector.tensor_tensor(
                        out=eg_s[:], in0=selfr[:, j, 0:1],
                        in1=alpha[:, nch + j, :], op=mybir.AluOpType.mult)
                cjs = (list(range(lo0, lo0 + lcnt))
                       + list(range(lw + hi0, lw + hi0 + hcnt)))
                for i, cj in enumerate(cjs):
                    nc.tensor.matmul(agg[:, j, :], lhsT=s_v[:, cj, :],
                                     rhs=eg[:, i, :], start=(i == 0), stop=False)
                nc.tensor.matmul(agg[:, j, :], lhsT=ident8[:],
                                 rhs=eg_s[:], start=(nch_t == 0), stop=True)

            # ---- epilogue
            if last:
                for j in range(gt):
                    t = t0 + j
                    nc.vector.tensor_scalar(
                        out=obuf[:, t:t + 1], in0=agg[:, j, :],
                        scalar1=float(inv_as3), scalar2=float(b3),
                        op0=mybir.AluOpType.mult, op1=mybir.AluOpType.add)
                continue

            yT = psb.tile([P, gt, 2, P], F32, space="PSUM", tag="yT")
            for j in range(gt):
                xn = sb.tile([P, F], FP16, tag="xn")
                nc.scalar.copy(xn[:], agg[:, j, :])
                for k in range(2):
                    xT_ps = psb.tile([P, P], FP16, space="PSUM", tag="xT")
                    nc.tensor.transpose(xT_ps[:], xn[:, k * P:(k + 1) * P],
                                        identb[:])
                    xTs = sb.tile([P, P], FP16, tag="xTs")
                    nc.scalar.copy(xTs[:], xT_ps[:])
                    nc.tensor.matmul(yT[:, j, k, :], lhsT=minv_t[:, k, :],
                                     rhs=xTs[:], start=True, stop=True)
            xe_g = sb.tile([P, gt, 2, P], FP16, tag="xe")
            for k in range(2):
                p_k = sb.tile([P, gt, P], FP16, tag="pk")
                nc.scalar.activation(p_k[:], yT[:, :, k, :],
                                     mybir.ActivationFunctionType.Relu,
                                     bias=bias_t[:, k:k + 1])
                m_k = sb.tile([P, gt, P], F32, tag="mk")
                nc.scalar.activation(m_k[:], yT[:, :, k, :],
                                     mybir.ActivationFunctionType.Relu,
                                     bias=nbias_t[:, k:k + 1], scale=-1.0)
                q_k = sb.tile([P, gt, P], FP16, tag="qk")
                nc.scalar.activation(q_k[:], m_k[:],
                                     mybir.ActivationFunctionType.Exp,
                                     scale=-1.0)
                nc.vector.tensor_tensor(out=xe_g[:, :, k, :], in0=p_k[:],
                                        in1=q_k[:], op=mybir.AluOpType.add)
            trow = sb.tile([P, gt, ROW if layer == 1 else ROW3], FP16, tag="trow")
            if layer == 2:
                nc.vector.memset(trow[:], 0.0)
            for j in range(gt):
                if layer == 1:
                    h_ps = psb.tile([P, naug], F32, space="PSUM", tag="hps")
                else:
                    h_ps = ps.tile([P, 2 + 40], F32, space="PSUM", tag="hps")
                for k in range(2):
                    nc.tensor.matmul(h_ps[:, 0:naug], lhsT=xe_g[:, j, k, :],
                                     rhs=waug_t[:, k, :], start=(k == 0),
                                     stop=(k == 1))
                if layer == 1:
                    nc.scalar.copy(trow[:, j, :], h_ps[:])
                    # overwrite as/ad slots with the exact (-1-corrected) values
                    nc.vector.tensor_tensor(
                        out=trow[:, j, :].rearrange("p (h c) -> p h c",
                                                    h=H)[:, :, 0:2],
                        in0=h_ps[:].rearrange("p (h c) -> p h c", h=H)[:, :, 0:2],
                        in1=wc_t[:].rearrange("p (h c) -> p h c", h=H),
                        op=mybir.AluOpType.subtract)
                else:
                    nc.vector.tensor_tensor(
                        out=trow[:, j, 0:2], in0=h_ps[:, 0:2], in1=wc_t[:],
                        op=mybir.AluOpType.subtract)
                    # replicate this tile's [as3, ad3] to its out-edge slots
                    t = t0 + j
                    och_t = int(OCH[t])
                    lo = int(oc0[t]) - go0
                    eps_v = h_ps[:, 2:2 + och_t * 2].rearrange(
                        "p (c v) -> p c v", v=2)
                    for i_oc in range(och_t):
                        nc.tensor.matmul(eps_v[:, i_oc, :],
                                         lhsT=ssrc_v[:, lo + i_oc, :],
                                         rhs=trow[:, j, 0:2],
                                         start=True, stop=True,
                                         skip_group_check=True)
                    nc.vector.tensor_copy(eout_sb[:, lo:lo + och_t, :], eps_v)
            nc.scalar.dma_start(
                tabout[t0 * P:(t0 + gt) * P, :].rearrange("(b p) f -> p b f", p=P),
                trow[:])
            if layer == 2:
                nc.scalar.dma_start(
                    eout[:, go0 * 2:go1 * 2].rearrange("p (c v) -> p c v", v=2),
                    eout_sb[:])
        if last:
            nc.scalar.dma_start(
                outv.rearrange("(t p) o -> p t o", p=P).squeeze(-1), obuf[:])
    nc.compile()
    return pr


# --------------------------------------------------------------- the kernel

LAST_TIMES = {}


def _run(pr, in_maps, tag=None):
    if tag is not None:
        try:
            from concourse.timeline_sim import TimelineSim
            LAST_TIMES[tag] = TimelineSim(pr.nc, trace=False).simulate() / 1e9
        except Exception:
            pass
    res = bass_utils.run_bass_kernel_spmd(
        pr.nc, in_maps, core_ids=list(range(N_CORES)))
    return res.results


def _pad_rows(a, n):
    out = np.zeros((n,) + a.shape[1:], a.dtype)
    out[:len(a)] = a
    return out


def kernel(x, edge_index, W1, a_src1, a_dst1, b1, W2, a_src2, a_dst2, b2,
           W3, a_src3, a_dst3, b3):
    x = np.asarray(x, np.float32)
    ei = np.asarray(edge_index)
    src = ei[0].astype(np.int64)
    dst = ei[1].astype(np.int64)

    pos = _balance_perm(dst)
    node_of_pos = np.empty(N_NODES, np.int64)
    node_of_pos[pos] = np.arange(N_NODES)
    sch = build_schedule(src, dst, pos)

    W1 = np.asarray(W1, np.float64)
    W2 = np.asarray(W2, np.float64)
    W3 = np.asarray(W3, np.float64)
    M1 = _rot_M(np.asarray(a_src1), np.asarray(a_dst1))
    M2 = _rot_M(np.asarray(a_src2), np.asarray(a_dst2))
    BD1, BD2 = _blockdiag(M1), _blockdiag(M2)
    W1rot = (W1 @ BD1).astype(np.float32)
    W2rot = (W2 @ BD2).astype(np.float32)
    a_s3 = float(np.asarray(a_src3).reshape(-1)[0])
    a_d3 = float(np.asarray(a_dst3).reshape(-1)[0])
    W3aug = np.concatenate([W3 * a_s3, W3 * a_d3], 1).astype(np.float32)

    Minv1 = _blockdiag([np.linalg.inv(m) for m in M1])
    Minv2 = _blockdiag([np.linalg.inv(m) for m in M2])
    minv1_t = np.stack([Minv1[k * P:(k + 1) * P, k * P:(k + 1) * P]
                        for k in range(2)]).transpose(1, 0, 2)
    minv2_t = np.stack([Minv2[k * P:(k + 1) * P, k * P:(k + 1) * P]
                        for k in range(2)]).transpose(1, 0, 2)

    w2row = W2rot.sum(0).astype(np.float64)            # ones @ W2rot
    w3row = W3aug.sum(0).astype(np.float64)
    wc2 = np.tile(w2row[SLOTS01].astype(np.float32), (P, 1))
    wc3 = np.tile(w3row.astype(np.float32), (P, 1))
    w2m = w2row.copy()
    w2m[SLOTS01] = 0.0
    b1v = np.asarray(b1, np.float64)
    b2v = np.asarray(b2, np.float64)
    b1_eff = b1v
    b2_eff = b2v - (w2m @ Minv2)
    b1T = b1_eff.astype(np.float32).reshape(2, P).T.copy()
    b2T = b2_eff.astype(np.float32).reshape(2, P).T.copy()

    bf = np.float16
    consts1 = dict(waug=W2rot.astype(bf), wc=wc2, minv=minv1_t.astype(bf),
                   bias=b1T, nbias=np.ascontiguousarray(-b1T))
    consts2 = dict(waug=W3aug.astype(bf), wc=wc3, minv=minv2_t.astype(bf),
                   bias=b2T, nbias=np.ascontiguousarray(-b2T))

    xp = x[node_of_pos]                                 # permuted rows

    prA = build_launch_A()
    xp16 = xp.astype(np.float16)
    W1rot16 = W1rot.astype(np.float16)
    inA = [dict(x=_pad_rows(xp16[c * NS:(c + 1) * NS], NSP), w1=W1rot16)
           for c in range(N_CORES)]
    resA = _run(prA, inA, tag="A")
    tab1 = np.ascontiguousarray(
        np.concatenate([resA[c]["tab"][:NS] for c in range(N_CORES)], 0))

    prB = build_launch_agg(sch, 1)
    inB = [dict(table=tab1, mytab=_pad_rows(tab1[c * NS:(c + 1) * NS], NSP),
                blob=sch["blob"][c], **consts1) for c in range(N_CORES)]
    resB = _run(prB, inB, tag="B")
    tab2 = np.ascontiguousarray(
        np.concatenate([resB[c]["tabout"][:NS] for c in range(N_CORES)], 0))

    prC = build_launch_agg(sch, 2)
    inC = [dict(table=tab2, mytab=_pad_rows(tab2[c * NS:(c + 1) * NS], NSP),
                blob=sch["blob"][c], ssrc=sch["ssrc"][c], **consts2)
           for c in range(N_CORES)]
    resC = _run(prC, inC, tag="C")
    tab3 = np.ascontiguousarray(
        np.concatenate([resC[c]["tabout"][:NS] for c in range(N_CORES)], 0))

    # halo exchange of pushed per-edge as3 values (reorder only)
    s_core, d_core = sch["s_core"], sch["d_core"]
    opos, dch, dsl = sch["opos"], sch["dch"], sch["dsl"]
    E = len(s_core)
    vals = np.zeros(E, np.float16)
    for c in range(N_CORES):
        msk = s_core == c
        arr = np.asarray(resC[c]["eout"]).reshape(P, sch["OTOT"], 2)
        vals[msk] = arr[opos[msk] % P, opos[msk] // P, 0]
    edata = np.zeros((N_CORES, P, sch["TOTCH"]), np.float16)
    for c in range(N_CORES):
        msk = d_core == c
        edata[c][dsl[msk], dch[msk]] = vals[msk]

    prD = build_launch_agg(sch, 3, b3=float(np.asarray(b3).reshape(-1)[0]),
                           inv_as3=1.0 / a_s3)
    inD = [dict(table=tab3, mytab=_pad_rows(tab3[c * NS:(c + 1) * NS], NSP),
                blob=sch["blob"][c], edata=edata[c]) for c in range(N_CORES)]
    resD = _run(prD, inD, tag="D")
    outp = np.concatenate([resD[c]["outv"][:NS] for c in range(N_CORES)], 0)
    out = outp[pos]                                     # back to node order
    return np.ascontiguousarray(out.astype(np.float32))
